# revision 10
# baseline (speedup 1.0000x reference)
"""Trainium2 Bass kernel for nn_CLIP_MINN_88210038326221.

Computes, for N=16384 samples x with h=zeros(2):
    x2 = mono(0, x);  y1 = mono(1, x);  y2 = mono(2, x2)
where mono(k, x) integrates elu(MLP_k(cat(t, 0, 0)))+1 over t in [0, x] via
Clenshaw-Curtis quadrature, then applies the (constant, because h=0)
conditioner affine: out = exp(c1_k) * z + c0_k.

Because h is always zero, every output is a scalar 1-D function of the
single input x. The kernel therefore:
  1. evaluates the full mono chain on a G=256-point uniform grid covering
     the x range (CC quadrature with 8 steps -- the functions are integrals
     of positive integrands and extremely smooth; ~1e-4 vs the reference's
     100-step rule),
  2. converts the grid values into piecewise-linear coefficients in the
     relu basis  y(x) = sum_i c_i * relu(x - bp_i)  via a constant
     second-difference matrix (on-device matmul; the matrix also carries a
     -h^2 f''/16 anti-bowing filter that halves the interp error),
  3. evaluates both outputs for all queries with three accumulating
     [128]-chunk f32 matmuls per 512-query tile.

Grid evaluation per weight set k (identical math to the direct kernel):
  t[g,s] = b[g] * c[s],  c[s] = (cos(s*pi/8)+1)/2
  a0 = relu(w0 t + b0)        -> K=2 matmul vs [t; 1] rows
  a1 = relu(W1 a0 + b1)       -> K=100 matmul, bias+relu in DVE
  a2 = relu(W2' a1 + b2')     -> zero row + bias 1 gives a free ones channel
  y3 = w3 . a2 + b3           -> lhsT = a2 chunk [101,128], rhs N=2
  sum_s ccw_s*(elu(y3)+1) = sum_s relu(ccw_s*y3') + sum_s ccw_s*exp(min(y3',0))
  out = exp(c1)*z + c0,  z = 0.5 * x * sum_s ccw_s*dz
mono0 and mono1 (and mono2's two grid blocks) are emitted as two
interleaved instruction streams so the PE never stalls on the
activation chain. Inputs arrive in three packed blob DMAs to keep the
serial DMA-issue cost off the critical path.
All 8 cores run the identical grid evaluation (replicated); the 16384
queries are sharded 2048 per core for the interpolation stage.
"""

import contextlib

import numpy as np

import concourse.bacc as bacc
import concourse.bass as bass
import concourse.mybir as mybir
import concourse.tile as tile
from concourse.bass_utils import run_bass_kernel_spmd
from concourse.masks import make_identity

F32 = mybir.dt.float32
F16 = mybir.dt.float16

N_CORES = 8
N_FULL = 16384
N_LOC = N_FULL // N_CORES      # 2048 queries per core
P = 128
G = 256                        # grid points (2 blocks of 128)
GBLK = G // P                  # 2
S_STEPS = 8                    # CC quadrature steps on the grid
S_REAL = S_STEPS + 1           # 9 quadrature points
S = 12                         # padded to multiple of 4
TILE_F = 512                   # free-dim tile (4 s-chunks of 128)
F_G = S * P                    # 1536 free per grid block
NT = F_G // TILE_F             # 3 tiles per block
NBP = 384                      # padded relu-basis breakpoints (3 chunks)
NCH = NBP // P                 # 3 breakpoint chunks
QT = N_LOC // TILE_F           # 4 query tiles
H_DIM = 2

# blob column layouts (elements)
B2_W0, B2_T1, B2_CW0 = 0, 300, 300 + GBLK * F_G
B2W = B2_CW0 + F_G                                   # [2, 4908] f16
BA_W1, BA_W2, BA_W3 = 0, 300, 603
BAW = BA_W3 + 3 * 2 * S                              # [101, 675] f16
BF_G, BF_B1, BF_B2, BF_CCW, BF_AG, BF_BP, BF_DT = 0, 2, 5, 8, 8 + S, 14 + S, 17 + S
BFW = BF_DT + 4 * P                                  # [128, 541] f32


def _cc_quadrature(nb):
    lam = np.arange(nb + 1).reshape(-1, 1).astype(np.float64)
    lam = np.cos((lam @ lam.T) * np.pi / nb)
    lam[:, 0] = 0.5
    lam[:, -1] = 0.5 * lam[:, -1]
    lam = lam * 2.0 / nb
    W = np.arange(nb + 1).reshape(-1, 1).astype(np.float64)
    W[np.arange(1, nb + 1, 2)] = 0.0
    W = 2.0 / (1.0 - W ** 2)
    W[0] = 1.0
    W[np.arange(1, nb + 1, 2)] = 0.0
    cc_w = (lam.T @ W).flatten()
    steps = np.cos(np.arange(nb + 1) * np.pi / nb)
    return cc_w, steps


CC_W, CC_STEPS = _cc_quadrature(S_STEPS)
C_PAD = np.zeros(S, np.float64)
C_PAD[:S_REAL] = (CC_STEPS + 1.0) * 0.5
CCW_PAD = np.zeros(S, np.float32)
CCW_PAD[:S_REAL] = CC_W


def build_program():
    nc = bacc.Bacc("TRN2", target_bir_lowering=False, debug=False)

    d_b2 = nc.dram_tensor("blob2", [2, B2W], F16, kind="ExternalInput").ap()
    d_ba = nc.dram_tensor("blob101", [101, BAW], F16, kind="ExternalInput").ap()
    d_bf = nc.dram_tensor("blob128", [P, BFW], F32, kind="ExternalInput").ap()
    d_xq = nc.dram_tensor("xq", [N_LOC], F32, kind="ExternalInput").ap()
    d_y = nc.dram_tensor("y", [2, N_LOC], F32, kind="ExternalOutput").ap()

    with tile.TileContext(nc) as tc, contextlib.ExitStack() as ctx:
        singles = ctx.enter_context(tc.tile_pool(name="singles", bufs=1))
        apool = ctx.enter_context(tc.tile_pool(name="apool", bufs=3))
        tailp = ctx.enter_context(tc.tile_pool(name="tailp", bufs=2))
        smallp = ctx.enter_context(tc.tile_pool(name="smallp", bufs=4))
        ppool = ctx.enter_context(tc.tile_pool(name="ppool", bufs=2, space="PSUM"))
        uvpool = ctx.enter_context(tc.tile_pool(name="uvpool", bufs=2, space="PSUM"))

        # ---- packed input DMAs (issue order = priority) ----
        blob2 = singles.tile([2, B2W], F16, tag="blob2")
        nc.sync.dma_start(out=blob2, in_=d_b2)
        bloba = singles.tile([101, BAW], F16, tag="bloba")
        nc.sync.dma_start(out=bloba, in_=d_ba)
        blobf = singles.tile([P, BFW], F32, tag="blobf")
        nc.sync.dma_start(out=blobf, in_=d_bf)
        xbroad = singles.tile([P, N_LOC], F32, tag="xbroad")
        nc.sync.dma_start(out=xbroad, in_=bass.AP(
            tensor=d_xq.tensor, offset=d_xq.offset,
            ap=[[0, P], d_xq.ap[0]]))

        w0b2 = blob2[:, B2_W0:B2_W0 + 300]
        t1v = [blob2[:, B2_T1 + b * F_G:B2_T1 + (b + 1) * F_G]
               for b in range(GBLK)]
        cw0 = blob2[:, B2_CW0:B2_CW0 + F_G]
        w1t = bloba[0:100, BA_W1:BA_W1 + 300]
        w2t = bloba[0:100, BA_W2:BA_W2 + 303]
        w3cc = bloba[:, BA_W3:BA_W3 + 3 * 2 * S]
        gcol = blobf[:, BF_G:BF_G + GBLK]
        b1 = blobf[0:100, BF_B1:BF_B1 + 3]
        b2p = blobf[0:101, BF_B2:BF_B2 + 3]
        ccwb = blobf[:, BF_CCW:BF_CCW + S]
        alphag = blobf[:, BF_AG:BF_AG + 6]
        bpcol = blobf[:, BF_BP:BF_BP + NCH]
        dtb = blobf[:, BF_DT:BF_DT + 4 * P]

        # ---- persistent SBUF ----
        ident = singles.tile([P, P], F32, tag="ident")
        make_identity(nc, ident)
        xx2 = singles.tile([2, G], F16, tag="xx2")
        nc.vector.memset(xx2, 1.0)
        x2t = singles.tile([GBLK, P], F32, tag="x2t")
        x2th = singles.tile([GBLK, P], F16, tag="x2th")
        x2col = singles.tile([P, GBLK], F32, tag="x2col")
        ygcol = singles.tile([P, 2 * GBLK], F32, tag="ygcol")
        csb = singles.tile([P, 2 * NCH], F32, tag="csb")
        you = singles.tile([2, N_LOC], F32, tag="you")
        r_acc = [singles.tile([P, GBLK], F32, tag=f"racc{k}", name=f"racc{k}")
                 for k in range(3)]
        # relu-basis tiles relu(xq - bp) on the otherwise-idle Pool engine;
        # they only depend on xbroad so they fill gaps during grid eval.
        rch = [singles.tile([P, TILE_F], F32, tag=f"rch{ti}_{j}",
                            name=f"rch{ti}_{j}")
               for ti in range(QT) for j in range(NCH)]
        for ti in range(QT):
            xsl = xbroad[:, ti * TILE_F:(ti + 1) * TILE_F]
            for j in range(NCH):
                nc.gpsimd.tensor_scalar(
                    out=rch[ti * NCH + j], in0=xsl,
                    scalar1=bpcol[:, j:j + 1], scalar2=0.0,
                    op0=mybir.AluOpType.subtract, op1=mybir.AluOpType.max)

        def uv_even_odd(uv):
            step = uv.ap[1][0]
            even = bass.AP(tensor=uv.tensor, offset=uv.offset,
                           ap=[uv.ap[0], [2 * step, S]])
            odd = bass.AP(tensor=uv.tensor, offset=uv.offset + step,
                          ap=[uv.ap[0], [2 * step, S]])
            return even, odd

        def mono_stream(k, b, t1_ap, a2_on_dve):
            """Generator: one (weight-set k, block b) pass, yielding after
            each instruction so two streams can interleave."""
            sfx = f"_{k}_{b}"
            uv = uvpool.tile([P, 2 * S], F32, tag="uv", name="uv" + sfx)
            for i in range(NT):
                a0ps = ppool.tile([P, TILE_F], F32, tag="a0ps",
                                  name=f"a0ps{sfx}_{i}")
                if t1_ap is not None:
                    nc.tensor.matmul(
                        a0ps[0:100, :], lhsT=w0b2[:, k * 100:(k + 1) * 100],
                        rhs=t1_ap[:, i * TILE_F:(i + 1) * TILE_F],
                        start=True, stop=True)
                    yield
                else:
                    for c in range(4):
                        s = 4 * i + c
                        nc.tensor.matmul(
                            a0ps[:, c * P:(c + 1) * P],
                            lhsT=cw0[:, s * P:(s + 1) * P],
                            rhs=xx2[:, b * P:(b + 1) * P],
                            start=True, stop=True)
                        yield
                a0sb = apool.tile([100, TILE_F], F16, tag="a0sb",
                                  name=f"a0sb{sfx}_{i}")
                nc.scalar.activation(out=a0sb, in_=a0ps[0:100, :],
                                     func=mybir.ActivationFunctionType.Relu,
                                     bias=0.0, scale=1.0)
                yield
                a1ps = ppool.tile([100, TILE_F], F32, tag="a1ps",
                                  name=f"a1ps{sfx}_{i}")
                nc.tensor.matmul(a1ps, lhsT=w1t[:, k * 100:(k + 1) * 100],
                                 rhs=a0sb, start=True, stop=True)
                yield
                a1sb = apool.tile([100, TILE_F], F16, tag="a1sb",
                                  name=f"a1sb{sfx}_{i}")
                nc.vector.tensor_scalar(out=a1sb, in0=a1ps,
                                        scalar1=b1[:, k:k + 1], scalar2=0.0,
                                        op0=mybir.AluOpType.add,
                                        op1=mybir.AluOpType.max)
                yield
                a2ps = ppool.tile([101, TILE_F], F32, tag="a2ps",
                                  name=f"a2ps{sfx}_{i}")
                nc.tensor.matmul(a2ps, lhsT=w2t[:, k * 101:(k + 1) * 101],
                                 rhs=a1sb, start=True, stop=True)
                yield
                a2sb = apool.tile([101, TILE_F], F16, tag="a2sb",
                                  name=f"a2sb{sfx}_{i}")
                if a2_on_dve:
                    nc.vector.tensor_scalar(out=a2sb, in0=a2ps,
                                            scalar1=b2p[:, k:k + 1],
                                            scalar2=0.0,
                                            op0=mybir.AluOpType.add,
                                            op1=mybir.AluOpType.max)
                else:
                    nc.scalar.activation(out=a2sb, in_=a2ps,
                                         func=mybir.ActivationFunctionType.Relu,
                                         bias=b2p[:, k:k + 1], scale=1.0)
                yield
                for c in range(4):
                    s = 4 * i + c
                    nc.tensor.matmul(
                        uv[:, 2 * s:2 * s + 2],
                        lhsT=a2sb[:, c * P:(c + 1) * P],
                        rhs=w3cc[:, k * 2 * S + 2 * s:k * 2 * S + 2 * s + 2],
                        start=True, stop=True)
                    yield
            # tail: r = sum_s relu(u) + sum_s ccw*exp(-relu(v))
            even, odd = uv_even_odd(uv)
            junk = tailp.tile([P, S], F32, tag="junk", name="junk" + sfx)
            r1 = smallp.tile([P, 1], F32, tag="r1", name="r1" + sfx)
            nc.scalar.activation(out=junk, in_=even,
                                 func=mybir.ActivationFunctionType.Relu,
                                 bias=0.0, scale=1.0, accum_out=r1[:, 0:1])
            yield
            wneg = tailp.tile([P, S], F32, tag="wneg", name="wneg" + sfx)
            nc.scalar.activation(out=wneg, in_=odd,
                                 func=mybir.ActivationFunctionType.Relu,
                                 bias=0.0, scale=1.0)
            yield
            e_t = tailp.tile([P, S], F32, tag="e_t", name="e_t" + sfx)
            nc.scalar.activation(out=e_t, in_=wneg,
                                 func=mybir.ActivationFunctionType.Exp,
                                 bias=0.0, scale=-1.0)
            yield
            g_t = tailp.tile([P, S], F32, tag="g_t", name="g_t" + sfx)
            nc.vector.tensor_mul(g_t, e_t, ccwb)
            yield
            r2 = smallp.tile([P, 1], F32, tag="r2", name="r2" + sfx)
            nc.vector.tensor_reduce(out=r2[:, 0:1], in_=g_t,
                                    axis=mybir.AxisListType.X,
                                    op=mybir.AluOpType.add)
            yield
            nc.vector.tensor_add(r_acc[k][:, b:b + 1], r1, r2)

        def drive(*gens):
            live = list(gens)
            while live:
                for g in list(live):
                    try:
                        next(g)
                    except StopIteration:
                        live.remove(g)

        def finalize(k, xcol_ap, out_ap):
            m = smallp.tile([P, GBLK], F32, tag="fin_m", name=f"fin_m{k}")
            nc.vector.tensor_mul(m, xcol_ap, r_acc[k])
            nc.vector.tensor_scalar(out=out_ap, in0=m,
                                    scalar1=alphag[:, k:k + 1],
                                    scalar2=alphag[:, 3 + k:4 + k],
                                    op0=mybir.AluOpType.mult,
                                    op1=mybir.AluOpType.add)

        # ---- grid eval: mono 0 and 1 interleaved per block ----
        for b in range(GBLK):
            drive(mono_stream(0, b, t1v[b], False),
                  mono_stream(1, b, t1v[b], True))

        # x2 = finalize(mono0); build xx2 row 0 = x2
        finalize(0, gcol, x2col)
        x2t_ps = uvpool.tile([GBLK, P], F32, tag="uv")
        nc.tensor.transpose(x2t_ps, x2col, ident)
        nc.scalar.copy(x2t, x2t_ps)
        nc.vector.tensor_copy(x2th, x2t)
        for b in range(GBLK):
            nc.sync.dma_start(out=xx2[0:1, b * P:(b + 1) * P],
                              in_=x2th[b:b + 1, :])

        # y1 grid values -> ygcol columns 0, 2 (chunk-major, func-minor)
        y1_ap = bass.AP(tensor=ygcol.tensor, offset=ygcol.offset,
                        ap=[ygcol.ap[0], [2 * ygcol.ap[1][0], GBLK]])
        finalize(1, gcol, y1_ap)

        # ---- mono 2 on x2 grid: both blocks interleaved ----
        drive(mono_stream(2, 0, None, False),
              mono_stream(2, 1, None, True))
        y2_ap = bass.AP(tensor=ygcol.tensor,
                        offset=ygcol.offset + ygcol.ap[1][0],
                        ap=[ygcol.ap[0], [2 * ygcol.ap[1][0], GBLK]])
        finalize(2, x2col, y2_ap)

        # ---- PWL coefficients: c = D @ ygrid (both funcs at once, N=2) ----
        # nonzero D^T blocks: (j=0,i=0), (j=1,i=0), (j=1,i=1), (j=2,i=1)
        blocks = [(0, 0, 0), (1, 1, 0), (2, 1, 1), (3, 2, 1)]
        cps = {}
        for j in range(NCH):
            cps[j] = uvpool.tile([P, 2], F32, tag="uv", name=f"cps{j}")
        for blk, j, i in blocks:
            first = (blk == 0 or blocks[blk - 1][1] != j)
            last = (blk == 3 or blocks[blk + 1][1] != j)
            nc.tensor.matmul(cps[j],
                             lhsT=dtb[:, blk * P:(blk + 1) * P],
                             rhs=ygcol[:, 2 * i:2 * i + 2],
                             start=first, stop=last)
        for j in range(NCH):
            nc.scalar.copy(csb[:, 2 * j:2 * j + 2], cps[j])

        # ---- interpolate queries: y[f, n] = sum_i c[f,i] relu(x_n - bp_i) ----
        for ti in range(QT):
            yps = ppool.tile([P, TILE_F], F32, tag="a0ps", name=f"yps{ti}")
            for j in range(NCH):
                nc.tensor.matmul(yps[0:2, :],
                                 lhsT=csb[:, 2 * j:2 * j + 2],
                                 rhs=rch[ti * NCH + j],
                                 start=(j == 0), stop=(j == NCH - 1))
            nc.scalar.copy(you[:, ti * TILE_F:(ti + 1) * TILE_F], yps[0:2, :])
        nc.sync.dma_start(out=d_y, in_=you)

    nc.compile()
    return nc


def host_grid_inputs(x_full, iws, ibs, hws, hbs):
    """Build the shared (grid + weights) input map; xq is added per core."""
    (iW0, iW1, iW2, iW3) = iws
    (ib0, ib1, ib2, ib3) = ibs

    xlo = float(x_full.min()) - 1e-3
    xhi = float(x_full.max()) + 1e-3
    b = np.linspace(xlo, xhi, G)
    h = b[1] - b[0]

    # f16 blob [2, B2W]: w0b2 | t-rows per block | cw0
    blob2 = np.empty((2, B2W), np.float16)
    w0col = iW0[:, :, 0]
    for k in range(3):
        blob2[0, k * 100:(k + 1) * 100] = w0col[k]
        blob2[1, k * 100:(k + 1) * 100] = ib0[k]
    bb = b.reshape(GBLK, P)
    grid = C_PAD[:, None] * bb[:, None, :]           # [blk, s, p]
    blob2[0, B2_T1:B2_CW0] = grid.reshape(GBLK * F_G)
    blob2[1, B2_T1:B2_CW0] = 1.0
    cw = np.zeros((2, F_G), np.float32)
    for s in range(S):
        cw[0, s * P:s * P + 100] = C_PAD[s] * w0col[2]
        cw[1, s * P:s * P + 100] = ib0[2]
    blob2[:, B2_CW0:] = cw

    # f16 blob [101, BAW]: w1t | w2t | w3cc
    bloba = np.zeros((101, BAW), np.float16)
    for k in range(3):
        bloba[0:100, BA_W1 + k * 100:BA_W1 + (k + 1) * 100] = iW1[k].T
        bloba[0:100, BA_W2 + k * 101:BA_W2 + k * 101 + 100] = iW2[k].T
    w3cc2 = np.zeros((101, 3 * 2 * S), np.float32)
    for k in range(3):
        col_p = np.concatenate([iW3[k, 0, :], [ib3[k, 0]]])
        for s in range(S):
            w3cc2[:, k * 2 * S + 2 * s] = CCW_PAD[s] * col_p
            w3cc2[:, k * 2 * S + 2 * s + 1] = -col_p
    bloba[:, BA_W3:] = w3cc2.astype(np.float16)

    # conditioner at h=0: alpha_k = 0.5*exp(c1_k), gamma_k = c0_k
    ag = np.empty(6, np.float64)
    for k in range(3):
        hh = np.zeros(H_DIM, np.float64)
        for li, (W, bv) in enumerate(zip(hws, hbs)):
            hh = W[k].astype(np.float64) @ hh + bv[k].astype(np.float64)
            if li < len(hws) - 1:
                hh = np.maximum(hh, 0.0)
        ag[k] = 0.5 * np.exp(hh[1])
        ag[3 + k] = hh[0]

    # breakpoints + filtered second-difference matrix D [NBP, G]
    bp = np.full(NBP, 1e9, np.float64)
    bp[0] = xlo - 2.0
    bp[1] = xlo - 1.0
    bp[2:2 + G - 1] = b[:G - 1]
    D = np.zeros((NBP, G))
    D[0, 0] = 1.0
    D[1, 0] = -1.0
    D[2, 0] = -1.0 / h
    D[2, 1] = 1.0 / h
    for j in range(1, G - 1):
        D[2 + j, j - 1] = 1.0 / h
        D[2 + j, j] = -2.0 / h
        D[2 + j, j + 1] = 1.0 / h
    T = np.zeros((G, G))
    for j in range(1, G - 1):
        T[j, j - 1] = 1.0
        T[j, j] = -2.0
        T[j, j + 1] = 1.0
    D = D @ (np.eye(G) - T / 16.0)

    # f32 blob [128, BFW]: gcol | b1 | b2p | ccwb | alphag | bpcol | dtb
    blobf = np.zeros((P, BFW), np.float32)
    blobf[:, BF_G:BF_G + GBLK] = b.reshape(GBLK, P).T
    blobf[0:100, BF_B1:BF_B1 + 3] = ib1.T
    for k in range(3):
        blobf[0:100, BF_B2 + k] = ib2[k]
        blobf[100, BF_B2 + k] = 1.0
    blobf[:, BF_CCW:BF_CCW + S] = CCW_PAD[None, :]
    blobf[:, BF_AG:BF_AG + 6] = ag[None, :]
    blobf[:, BF_BP:BF_BP + NCH] = bp.reshape(NCH, P).T
    for blk, (j, i) in enumerate(((0, 0), (1, 0), (1, 1), (2, 1))):
        blobf[:, BF_DT + blk * P:BF_DT + (blk + 1) * P] = \
            D[j * P:(j + 1) * P, i * P:(i + 1) * P].T

    return {"blob2": blob2, "blob101": bloba, "blob128": blobf}


def make_in_maps(logits_quality,
                 iW0, ib0, iW1, ib1, iW2, ib2, iW3, ib3,
                 hW0, hb0, hW1, hb1, hW2, hb2, hW3, hb3, **_):
    x = np.asarray(logits_quality, np.float32)
    iws = [np.asarray(a, np.float32) for a in (iW0, iW1, iW2, iW3)]
    ibs = [np.asarray(a, np.float32) for a in (ib0, ib1, ib2, ib3)]
    hws = [np.asarray(a, np.float32) for a in (hW0, hW1, hW2, hW3)]
    hbs = [np.asarray(a, np.float32) for a in (hb0, hb1, hb2, hb3)]
    shared = host_grid_inputs(x, iws, ibs, hws, hbs)
    in_maps = []
    for c in range(N_CORES):
        im = dict(shared)
        im["xq"] = np.ascontiguousarray(x[c * N_LOC:(c + 1) * N_LOC])
        in_maps.append(im)
    return x, in_maps


_PROGRAM_CACHE = {}


def get_program():
    if "nc" not in _PROGRAM_CACHE:
        _PROGRAM_CACHE["nc"] = build_program()
    return _PROGRAM_CACHE["nc"]


def kernel(logits_quality, nn_id,
           iW0, ib0, iW1, ib1, iW2, ib2, iW3, ib3,
           hW0, hb0, hW1, hb1, hW2, hb2, hW3, hb3):
    x, in_maps = make_in_maps(
        logits_quality,
        iW0, ib0, iW1, ib1, iW2, ib2, iW3, ib3,
        hW0, hb0, hW1, hb1, hW2, hb2, hW3, hb3)
    nc = get_program()
    res = run_bass_kernel_spmd(nc, in_maps, core_ids=list(range(N_CORES)))
    y1 = np.concatenate([r["y"][0] for r in res.results])
    y2 = np.concatenate([r["y"][1] for r in res.results])
    return (y1, y2, x)


# revision 15
# speedup vs baseline: 1.5379x; 1.5379x over previous
"""Trainium2 Bass kernel for nn_CLIP_MINN_88210038326221.

Computes, for N=16384 samples x with h=zeros(2):
    x2 = mono(0, x);  y1 = mono(1, x);  y2 = mono(2, x2)
where mono(k, x) integrates elu(MLP_k(cat(t, 0, 0)))+1 over t in [0, x] via
Clenshaw-Curtis quadrature, then applies the (constant, because h=0)
conditioner affine: out = exp(c1_k) * z + c0_k.

Because h is always zero, every output is a scalar 1-D function of the
single input x. The kernel therefore:
  1. evaluates the full mono chain on a G=256-point uniform grid covering
     the x range (CC quadrature with 8 steps -- the functions are integrals
     of positive integrands and extremely smooth; ~1e-4 vs the reference's
     100-step rule),
  2. converts the grid values into piecewise-linear coefficients in the
     relu basis  y(x) = sum_i c_i * relu(x - bp_i)  via a constant
     second-difference matrix (on-device matmul; the matrix also carries a
     -h^2 f''/16 anti-bowing filter that halves the interp error),
  3. evaluates both outputs for all queries with three accumulating
     [128]-chunk f32 matmuls per 512-query tile.

Grid evaluation per weight set k (identical math to the direct kernel):
  t[g,s] = b[g] * c[s],  c[s] = (cos(s*pi/8)+1)/2
  a0 = relu(w0 t + b0)        -> K=2 matmul vs [t; 1] rows
  a1 = relu(W1 a0 + b1)       -> K=100 matmul, bias+relu in DVE
  a2 = relu(W2' a1 + b2')     -> zero row + bias 1 gives a free ones channel
  y3 = w3 . a2 + b3           -> lhsT = a2 chunk [101,128], rhs N=2
  sum_s ccw_s*(elu(y3)+1) = sum_s relu(ccw_s*y3') + sum_s ccw_s*exp(min(y3',0))
  out = exp(c1)*z + c0,  z = 0.5 * x * sum_s ccw_s*dz
mono0 and mono1 (and mono2's two grid blocks) are emitted as two
interleaved instruction streams so the PE never stalls on the
activation chain. Inputs arrive in three packed blob DMAs to keep the
serial DMA-issue cost off the critical path.
All 8 cores run the identical grid evaluation (replicated); the 16384
queries are sharded 2048 per core for the interpolation stage.
"""

import contextlib

import numpy as np

import concourse.bacc as bacc
import concourse.bass as bass
import concourse.mybir as mybir
import concourse.tile as tile
from concourse.bass_utils import run_bass_kernel_spmd
from concourse.masks import make_identity

F32 = mybir.dt.float32
F16 = mybir.dt.float16

N_CORES = 8
N_FULL = 16384
N_LOC = N_FULL // N_CORES      # 2048 queries per core
P = 128
G = 256                        # grid points (2 blocks of 128)
GBLK = G // P                  # 2
S_STEPS = 8                    # CC quadrature steps on the grid
S_REAL = S_STEPS + 1           # 9 quadrature points
S = 12                         # padded to multiple of 4
TILE_F = 512                   # free-dim tile (4 s-chunks of 128)
F_G = S * P                    # 1536 free per grid block
NT = F_G // TILE_F             # 3 tiles per block
NBP = 384                      # padded relu-basis breakpoints (3 chunks)
NCH = NBP // P                 # 3 breakpoint chunks
QT = N_LOC // TILE_F           # 4 query tiles
H_DIM = 2

# blob column layouts (elements)
B2_W0, B2_T1, B2_CW0 = 0, 300, 300 + GBLK * F_G
B2W = B2_CW0 + F_G                                   # [2, 4908] f16
BA_W1, BA_W2, BA_W3 = 0, 300, 603
BAW = BA_W3 + 3 * 2 * S                              # [101, 675] f16
BF_G, BF_B1, BF_B2, BF_CCW, BF_AG, BF_BP, BF_BPN, BF_DT = (
    0, 2, 5, 8, 8 + S, 14 + S, 17 + S, 20 + S)
BFW = BF_DT + 4 * P                                  # [128, 544] f32


def _cc_quadrature(nb):
    lam = np.arange(nb + 1).reshape(-1, 1).astype(np.float64)
    lam = np.cos((lam @ lam.T) * np.pi / nb)
    lam[:, 0] = 0.5
    lam[:, -1] = 0.5 * lam[:, -1]
    lam = lam * 2.0 / nb
    W = np.arange(nb + 1).reshape(-1, 1).astype(np.float64)
    W[np.arange(1, nb + 1, 2)] = 0.0
    W = 2.0 / (1.0 - W ** 2)
    W[0] = 1.0
    W[np.arange(1, nb + 1, 2)] = 0.0
    cc_w = (lam.T @ W).flatten()
    steps = np.cos(np.arange(nb + 1) * np.pi / nb)
    return cc_w, steps


CC_W, CC_STEPS = _cc_quadrature(S_STEPS)
C_PAD = np.zeros(S, np.float64)
C_PAD[:S_REAL] = (CC_STEPS + 1.0) * 0.5
CCW_PAD = np.zeros(S, np.float32)
CCW_PAD[:S_REAL] = CC_W


def build_program():
    nc = bacc.Bacc("TRN2", target_bir_lowering=False, debug=False)

    d_b2 = nc.dram_tensor("blob2", [2, B2W], F16, kind="ExternalInput").ap()
    d_ba = nc.dram_tensor("blob101", [101, BAW], F16, kind="ExternalInput").ap()
    d_bf = nc.dram_tensor("blob128", [P, BFW], F32, kind="ExternalInput").ap()
    d_xq = nc.dram_tensor("xq", [N_LOC], F32, kind="ExternalInput").ap()
    d_y = nc.dram_tensor("y", [2, N_LOC], F32, kind="ExternalOutput").ap()

    with tile.TileContext(nc) as tc, contextlib.ExitStack() as ctx:
        singles = ctx.enter_context(tc.tile_pool(name="singles", bufs=1))
        apool = ctx.enter_context(tc.tile_pool(name="apool", bufs=3))
        tailp = ctx.enter_context(tc.tile_pool(name="tailp", bufs=2))
        smallp = ctx.enter_context(tc.tile_pool(name="smallp", bufs=4))
        ppool = ctx.enter_context(tc.tile_pool(name="ppool", bufs=2, space="PSUM"))
        uvpool = ctx.enter_context(tc.tile_pool(name="uvpool", bufs=2, space="PSUM"))

        # ---- packed input DMAs (issue order = priority) ----
        blob2 = singles.tile([2, B2W], F16, tag="blob2")
        nc.sync.dma_start(out=blob2, in_=d_b2)
        bloba = singles.tile([101, BAW], F16, tag="bloba")
        nc.sync.dma_start(out=bloba, in_=d_ba)
        blobf = singles.tile([P, BFW], F32, tag="blobf")
        nc.sync.dma_start(out=blobf, in_=d_bf)
        xbroad = singles.tile([P, N_LOC], F32, tag="xbroad")
        nc.sync.dma_start(out=xbroad, in_=bass.AP(
            tensor=d_xq.tensor, offset=d_xq.offset,
            ap=[[0, P], d_xq.ap[0]]))

        w0b2 = blob2[:, B2_W0:B2_W0 + 300]
        t1v = [blob2[:, B2_T1 + b * F_G:B2_T1 + (b + 1) * F_G]
               for b in range(GBLK)]
        cw0 = blob2[:, B2_CW0:B2_CW0 + F_G]
        w1t = bloba[0:100, BA_W1:BA_W1 + 300]
        w2t = bloba[0:100, BA_W2:BA_W2 + 303]
        w3cc = bloba[:, BA_W3:BA_W3 + 3 * 2 * S]
        gcol = blobf[:, BF_G:BF_G + GBLK]
        b1 = blobf[0:100, BF_B1:BF_B1 + 3]
        b2p = blobf[0:101, BF_B2:BF_B2 + 3]
        ccwb = blobf[:, BF_CCW:BF_CCW + S]
        alphag = blobf[:, BF_AG:BF_AG + 6]
        bpcol = blobf[:, BF_BP:BF_BP + NCH]
        bpncol = blobf[:, BF_BPN:BF_BPN + NCH]
        dtb = blobf[:, BF_DT:BF_DT + 4 * P]

        # ---- persistent SBUF ----
        ident = singles.tile([P, P], F32, tag="ident")
        make_identity(nc, ident)
        xx2 = singles.tile([2, G], F16, tag="xx2")
        nc.vector.memset(xx2, 1.0)
        x2t = singles.tile([GBLK, P], F32, tag="x2t")
        x2th = singles.tile([GBLK, P], F16, tag="x2th")
        x2col = singles.tile([P, GBLK], F32, tag="x2col")
        ygcol = singles.tile([P, 2 * GBLK], F32, tag="ygcol")
        csb = singles.tile([P, 2 * NCH], F32, tag="csb")
        you = singles.tile([2, N_LOC], F32, tag="you")
        r_acc = [singles.tile([P, GBLK], F32, tag=f"racc{k}", name=f"racc{k}")
                 for k in range(3)]
        # relu-basis tiles relu(xq - bp); emitted in batches between grid
        # phases (engines are in-order, so emitting them up front would
        # stall the queue on the large xbroad DMA).
        rch = [singles.tile([P, TILE_F], F32, tag=f"rch{ti}_{j}",
                            name=f"rch{ti}_{j}")
               for ti in range(QT) for j in range(NCH)]

        def emit_rch(lo, hi):
            for idx in range(lo, hi):
                ti, j = divmod(idx, NCH)
                xsl = xbroad[:, ti * TILE_F:(ti + 1) * TILE_F]
                if idx % 2 == 0:
                    nc.vector.tensor_scalar(
                        out=rch[idx], in0=xsl,
                        scalar1=bpcol[:, j:j + 1], scalar2=0.0,
                        op0=mybir.AluOpType.subtract, op1=mybir.AluOpType.max)
                else:
                    nc.scalar.activation(
                        out=rch[idx], in_=xsl,
                        func=mybir.ActivationFunctionType.Relu,
                        bias=bpncol[:, j:j + 1], scale=1.0)

        def uv_even_odd(uv):
            step = uv.ap[1][0]
            even = bass.AP(tensor=uv.tensor, offset=uv.offset,
                           ap=[uv.ap[0], [2 * step, S]])
            odd = bass.AP(tensor=uv.tensor, offset=uv.offset + step,
                          ap=[uv.ap[0], [2 * step, S]])
            return even, odd

        def mono_stream(k, b, t1_ap, a2_on_dve):
            """Generator: one (weight-set k, block b) pass, yielding after
            each instruction so two streams can interleave."""
            sfx = f"_{k}_{b}"
            uv = uvpool.tile([P, 2 * S], F32, tag="uv", name="uv" + sfx)
            for i in range(NT):
                a0ps = ppool.tile([P, TILE_F], F32, tag="a0ps",
                                  name=f"a0ps{sfx}_{i}")
                if t1_ap is not None:
                    nc.tensor.matmul(
                        a0ps[0:100, :], lhsT=w0b2[:, k * 100:(k + 1) * 100],
                        rhs=t1_ap[:, i * TILE_F:(i + 1) * TILE_F],
                        start=True, stop=True)
                    yield
                else:
                    for c in range(4):
                        s = 4 * i + c
                        nc.tensor.matmul(
                            a0ps[:, c * P:(c + 1) * P],
                            lhsT=cw0[:, s * P:(s + 1) * P],
                            rhs=xx2[:, b * P:(b + 1) * P],
                            start=True, stop=True)
                        yield
                a0sb = apool.tile([100, TILE_F], F16, tag="a0sb",
                                  name=f"a0sb{sfx}_{i}")
                nc.scalar.activation(out=a0sb, in_=a0ps[0:100, :],
                                     func=mybir.ActivationFunctionType.Relu,
                                     bias=0.0, scale=1.0)
                yield
                a1ps = ppool.tile([100, TILE_F], F32, tag="a1ps",
                                  name=f"a1ps{sfx}_{i}")
                nc.tensor.matmul(a1ps, lhsT=w1t[:, k * 100:(k + 1) * 100],
                                 rhs=a0sb, start=True, stop=True)
                yield
                a1sb = apool.tile([100, TILE_F], F16, tag="a1sb",
                                  name=f"a1sb{sfx}_{i}")
                nc.vector.tensor_scalar(out=a1sb, in0=a1ps,
                                        scalar1=b1[:, k:k + 1], scalar2=0.0,
                                        op0=mybir.AluOpType.add,
                                        op1=mybir.AluOpType.max)
                yield
                a2ps = ppool.tile([101, TILE_F], F32, tag="a2ps",
                                  name=f"a2ps{sfx}_{i}")
                nc.tensor.matmul(a2ps, lhsT=w2t[:, k * 101:(k + 1) * 101],
                                 rhs=a1sb, start=True, stop=True)
                yield
                a2sb = apool.tile([101, TILE_F], F16, tag="a2sb",
                                  name=f"a2sb{sfx}_{i}")
                if a2_on_dve:
                    nc.vector.tensor_scalar(out=a2sb, in0=a2ps,
                                            scalar1=b2p[:, k:k + 1],
                                            scalar2=0.0,
                                            op0=mybir.AluOpType.add,
                                            op1=mybir.AluOpType.max)
                else:
                    nc.scalar.activation(out=a2sb, in_=a2ps,
                                         func=mybir.ActivationFunctionType.Relu,
                                         bias=b2p[:, k:k + 1], scale=1.0)
                yield
                for c in range(4):
                    s = 4 * i + c
                    nc.tensor.matmul(
                        uv[:, 2 * s:2 * s + 2],
                        lhsT=a2sb[:, c * P:(c + 1) * P],
                        rhs=w3cc[:, k * 2 * S + 2 * s:k * 2 * S + 2 * s + 2],
                        start=True, stop=True)
                    yield
            # tail: r = sum_s relu(u) + sum_s ccw*exp(-relu(v))
            even, odd = uv_even_odd(uv)
            junk = tailp.tile([P, S], F32, tag="junk", name="junk" + sfx)
            r1 = smallp.tile([P, 1], F32, tag="r1", name="r1" + sfx)
            nc.scalar.activation(out=junk, in_=even,
                                 func=mybir.ActivationFunctionType.Relu,
                                 bias=0.0, scale=1.0, accum_out=r1[:, 0:1])
            yield
            wneg = tailp.tile([P, S], F32, tag="wneg", name="wneg" + sfx)
            nc.scalar.activation(out=wneg, in_=odd,
                                 func=mybir.ActivationFunctionType.Relu,
                                 bias=0.0, scale=1.0)
            yield
            e_t = tailp.tile([P, S], F32, tag="e_t", name="e_t" + sfx)
            nc.scalar.activation(out=e_t, in_=wneg,
                                 func=mybir.ActivationFunctionType.Exp,
                                 bias=0.0, scale=-1.0)
            yield
            g_t = tailp.tile([P, S], F32, tag="g_t", name="g_t" + sfx)
            nc.vector.tensor_mul(g_t, e_t, ccwb)
            yield
            r2 = smallp.tile([P, 1], F32, tag="r2", name="r2" + sfx)
            nc.vector.tensor_reduce(out=r2[:, 0:1], in_=g_t,
                                    axis=mybir.AxisListType.X,
                                    op=mybir.AluOpType.add)
            yield
            nc.vector.tensor_add(r_acc[k][:, b:b + 1], r1, r2)

        def drive(*gens):
            live = list(gens)
            while live:
                for g in list(live):
                    try:
                        next(g)
                    except StopIteration:
                        live.remove(g)

        def finalize(k, xcol_ap, out_ap):
            m = smallp.tile([P, GBLK], F32, tag="fin_m", name=f"fin_m{k}")
            nc.vector.tensor_mul(m, xcol_ap, r_acc[k])
            nc.vector.tensor_scalar(out=out_ap, in0=m,
                                    scalar1=alphag[:, k:k + 1],
                                    scalar2=alphag[:, 3 + k:4 + k],
                                    op0=mybir.AluOpType.mult,
                                    op1=mybir.AluOpType.add)

        # ---- grid eval: mono0 first (both blocks interleaved) so the
        # x2 chain and its DMA latency hide under mono1 ----
        drive(mono_stream(0, 0, t1v[0], False),
              mono_stream(0, 1, t1v[1], True))

        # x2 = finalize(mono0); build xx2 row 0 = x2
        finalize(0, gcol, x2col)
        x2t_ps = uvpool.tile([GBLK, P], F32, tag="uv")
        nc.tensor.transpose(x2t_ps, x2col, ident)
        nc.scalar.copy(x2t, x2t_ps)
        nc.vector.tensor_copy(x2th, x2t)
        for b in range(GBLK):
            nc.sync.dma_start(out=xx2[0:1, b * P:(b + 1) * P],
                              in_=x2th[b:b + 1, :])
        emit_rch(0, 6)

        # ---- mono 1 (both blocks interleaved) ----
        drive(mono_stream(1, 0, t1v[0], False),
              mono_stream(1, 1, t1v[1], True))
        # y1 grid values -> ygcol columns 0, 2 (chunk-major, func-minor)
        y1_ap = bass.AP(tensor=ygcol.tensor, offset=ygcol.offset,
                        ap=[ygcol.ap[0], [2 * ygcol.ap[1][0], GBLK]])
        finalize(1, gcol, y1_ap)
        emit_rch(6, 12)

        # ---- mono 2 on x2 grid: both blocks interleaved ----
        drive(mono_stream(2, 0, None, False),
              mono_stream(2, 1, None, True))
        y2_ap = bass.AP(tensor=ygcol.tensor,
                        offset=ygcol.offset + ygcol.ap[1][0],
                        ap=[ygcol.ap[0], [2 * ygcol.ap[1][0], GBLK]])
        finalize(2, x2col, y2_ap)

        # ---- PWL coefficients: c = D @ ygrid (both funcs at once, N=2) ----
        # nonzero D^T blocks: (j=0,i=0), (j=1,i=0), (j=1,i=1), (j=2,i=1)
        blocks = [(0, 0, 0), (1, 1, 0), (2, 1, 1), (3, 2, 1)]
        cps = {}
        for j in range(NCH):
            cps[j] = uvpool.tile([P, 2], F32, tag="uv", name=f"cps{j}")
        for blk, j, i in blocks:
            first = (blk == 0 or blocks[blk - 1][1] != j)
            last = (blk == 3 or blocks[blk + 1][1] != j)
            nc.tensor.matmul(cps[j],
                             lhsT=dtb[:, blk * P:(blk + 1) * P],
                             rhs=ygcol[:, 2 * i:2 * i + 2],
                             start=first, stop=last)
        for j in range(NCH):
            nc.scalar.copy(csb[:, 2 * j:2 * j + 2], cps[j])

        # ---- interpolate queries: y[f, n] = sum_i c[f,i] relu(x_n - bp_i) ----
        for ti in range(QT):
            yps = ppool.tile([P, TILE_F], F32, tag="a0ps", name=f"yps{ti}")
            for j in range(NCH):
                nc.tensor.matmul(yps[0:2, :],
                                 lhsT=csb[:, 2 * j:2 * j + 2],
                                 rhs=rch[ti * NCH + j],
                                 start=(j == 0), stop=(j == NCH - 1))
            nc.scalar.copy(you[:, ti * TILE_F:(ti + 1) * TILE_F], yps[0:2, :])
        nc.sync.dma_start(out=d_y, in_=you)

    nc.compile()
    return nc


def host_grid_inputs(x_full, iws, ibs, hws, hbs):
    """Build the shared (grid + weights) input map; xq is added per core."""
    (iW0, iW1, iW2, iW3) = iws
    (ib0, ib1, ib2, ib3) = ibs

    xlo = float(x_full.min()) - 1e-3
    xhi = float(x_full.max()) + 1e-3
    b = np.linspace(xlo, xhi, G)
    h = b[1] - b[0]

    # f16 blob [2, B2W]: w0b2 | t-rows per block | cw0
    blob2 = np.empty((2, B2W), np.float16)
    w0col = iW0[:, :, 0]
    for k in range(3):
        blob2[0, k * 100:(k + 1) * 100] = w0col[k]
        blob2[1, k * 100:(k + 1) * 100] = ib0[k]
    bb = b.reshape(GBLK, P)
    grid = C_PAD[:, None] * bb[:, None, :]           # [blk, s, p]
    blob2[0, B2_T1:B2_CW0] = grid.reshape(GBLK * F_G)
    blob2[1, B2_T1:B2_CW0] = 1.0
    cw = np.zeros((2, F_G), np.float32)
    for s in range(S):
        cw[0, s * P:s * P + 100] = C_PAD[s] * w0col[2]
        cw[1, s * P:s * P + 100] = ib0[2]
    blob2[:, B2_CW0:] = cw

    # f16 blob [101, BAW]: w1t | w2t | w3cc
    bloba = np.zeros((101, BAW), np.float16)
    for k in range(3):
        bloba[0:100, BA_W1 + k * 100:BA_W1 + (k + 1) * 100] = iW1[k].T
        bloba[0:100, BA_W2 + k * 101:BA_W2 + k * 101 + 100] = iW2[k].T
    w3cc2 = np.zeros((101, 3 * 2 * S), np.float32)
    for k in range(3):
        col_p = np.concatenate([iW3[k, 0, :], [ib3[k, 0]]])
        for s in range(S):
            w3cc2[:, k * 2 * S + 2 * s] = CCW_PAD[s] * col_p
            w3cc2[:, k * 2 * S + 2 * s + 1] = -col_p
    bloba[:, BA_W3:] = w3cc2.astype(np.float16)

    # conditioner at h=0: alpha_k = 0.5*exp(c1_k), gamma_k = c0_k
    ag = np.empty(6, np.float64)
    for k in range(3):
        hh = np.zeros(H_DIM, np.float64)
        for li, (W, bv) in enumerate(zip(hws, hbs)):
            hh = W[k].astype(np.float64) @ hh + bv[k].astype(np.float64)
            if li < len(hws) - 1:
                hh = np.maximum(hh, 0.0)
        ag[k] = 0.5 * np.exp(hh[1])
        ag[3 + k] = hh[0]

    # breakpoints + filtered second-difference matrix D [NBP, G]
    bp = np.full(NBP, 1e9, np.float64)
    bp[0] = xlo - 2.0
    bp[1] = xlo - 1.0
    bp[2:2 + G - 1] = b[:G - 1]
    D = np.zeros((NBP, G))
    D[0, 0] = 1.0
    D[1, 0] = -1.0
    D[2, 0] = -1.0 / h
    D[2, 1] = 1.0 / h
    for j in range(1, G - 1):
        D[2 + j, j - 1] = 1.0 / h
        D[2 + j, j] = -2.0 / h
        D[2 + j, j + 1] = 1.0 / h
    T = np.zeros((G, G))
    for j in range(1, G - 1):
        T[j, j - 1] = 1.0
        T[j, j] = -2.0
        T[j, j + 1] = 1.0
    D = D @ (np.eye(G) - T / 16.0)

    # f32 blob [128, BFW]: gcol | b1 | b2p | ccwb | alphag | bpcol | dtb
    blobf = np.zeros((P, BFW), np.float32)
    blobf[:, BF_G:BF_G + GBLK] = b.reshape(GBLK, P).T
    blobf[0:100, BF_B1:BF_B1 + 3] = ib1.T
    for k in range(3):
        blobf[0:100, BF_B2 + k] = ib2[k]
        blobf[100, BF_B2 + k] = 1.0
    blobf[:, BF_CCW:BF_CCW + S] = CCW_PAD[None, :]
    blobf[:, BF_AG:BF_AG + 6] = ag[None, :]
    blobf[:, BF_BP:BF_BP + NCH] = bp.reshape(NCH, P).T
    blobf[:, BF_BPN:BF_BPN + NCH] = -bp.reshape(NCH, P).T
    for blk, (j, i) in enumerate(((0, 0), (1, 0), (1, 1), (2, 1))):
        blobf[:, BF_DT + blk * P:BF_DT + (blk + 1) * P] = \
            D[j * P:(j + 1) * P, i * P:(i + 1) * P].T

    return {"blob2": blob2, "blob101": bloba, "blob128": blobf}


def make_in_maps(logits_quality,
                 iW0, ib0, iW1, ib1, iW2, ib2, iW3, ib3,
                 hW0, hb0, hW1, hb1, hW2, hb2, hW3, hb3, **_):
    x = np.asarray(logits_quality, np.float32)
    iws = [np.asarray(a, np.float32) for a in (iW0, iW1, iW2, iW3)]
    ibs = [np.asarray(a, np.float32) for a in (ib0, ib1, ib2, ib3)]
    hws = [np.asarray(a, np.float32) for a in (hW0, hW1, hW2, hW3)]
    hbs = [np.asarray(a, np.float32) for a in (hb0, hb1, hb2, hb3)]
    shared = host_grid_inputs(x, iws, ibs, hws, hbs)
    in_maps = []
    for c in range(N_CORES):
        im = dict(shared)
        im["xq"] = np.ascontiguousarray(x[c * N_LOC:(c + 1) * N_LOC])
        in_maps.append(im)
    return x, in_maps


_PROGRAM_CACHE = {}


def get_program():
    if "nc" not in _PROGRAM_CACHE:
        _PROGRAM_CACHE["nc"] = build_program()
    return _PROGRAM_CACHE["nc"]


def kernel(logits_quality, nn_id,
           iW0, ib0, iW1, ib1, iW2, ib2, iW3, ib3,
           hW0, hb0, hW1, hb1, hW2, hb2, hW3, hb3):
    x, in_maps = make_in_maps(
        logits_quality,
        iW0, ib0, iW1, ib1, iW2, ib2, iW3, ib3,
        hW0, hb0, hW1, hb1, hW2, hb2, hW3, hb3)
    nc = get_program()
    res = run_bass_kernel_spmd(nc, in_maps, core_ids=list(range(N_CORES)))
    y1 = np.concatenate([r["y"][0] for r in res.results])
    y2 = np.concatenate([r["y"][1] for r in res.results])
    return (y1, y2, x)


# revision 20
# speedup vs baseline: 1.5427x; 1.0032x over previous
"""Trainium2 Bass kernel for nn_CLIP_MINN_88210038326221.

Computes, for N=16384 samples x with h=zeros(2):
    x2 = mono(0, x);  y1 = mono(1, x);  y2 = mono(2, x2)
where mono(k, x) integrates elu(MLP_k(cat(t, 0, 0)))+1 over t in [0, x] via
Clenshaw-Curtis quadrature, then applies the (constant, because h=0)
conditioner affine: out = exp(c1_k) * z + c0_k.

Because h is always zero, every output is a scalar 1-D function of the
single input x. The kernel therefore:
  1. evaluates the full mono chain on a G=256-point uniform grid covering
     the x range (CC quadrature with 8 steps -- the functions are integrals
     of positive integrands and extremely smooth; ~1e-4 vs the reference's
     100-step rule),
  2. converts the grid values into piecewise-linear coefficients in the
     relu basis  y(x) = sum_i c_i * relu(x - bp_i)  via a constant
     second-difference matrix (on-device matmul; the matrix also carries a
     -h^2 f''/16 anti-bowing filter that halves the interp error),
  3. evaluates both outputs for all queries with three accumulating
     [128]-chunk f32 matmuls per 512-query tile.

Grid evaluation per weight set k (identical math to the direct kernel):
  t[g,s] = b[g] * c[s],  c[s] = (cos(s*pi/8)+1)/2
  a0 = relu(w0 t + b0)        -> K=2 matmul vs [t; 1] rows
  a1 = relu(W1 a0 + b1)       -> K=100 matmul, bias+relu in DVE
  a2 = relu(W2' a1 + b2')     -> zero row + bias 1 gives a free ones channel
  y3 = w3 . a2 + b3           -> lhsT = a2 chunk [101,128], rhs N=2
  sum_s ccw_s*(elu(y3)+1) = sum_s relu(ccw_s*y3') + sum_s ccw_s*exp(min(y3',0))
  out = exp(c1)*z + c0,  z = 0.5 * x * sum_s ccw_s*dz
mono0 and mono1 (and mono2's two grid blocks) are emitted as two
interleaved instruction streams so the PE never stalls on the
activation chain. Inputs arrive in three packed blob DMAs to keep the
serial DMA-issue cost off the critical path.
All 8 cores run the identical grid evaluation (replicated); the 16384
queries are sharded 2048 per core for the interpolation stage.
"""

import contextlib

import numpy as np

import concourse.bacc as bacc
import concourse.bass as bass
import concourse.mybir as mybir
import concourse.tile as tile
from concourse.bass_utils import run_bass_kernel_spmd
from concourse.masks import make_identity

F32 = mybir.dt.float32
F16 = mybir.dt.float16

N_CORES = 8
N_FULL = 16384
N_LOC = N_FULL // N_CORES      # 2048 queries per core
P = 128
G = 256                        # grid points (2 blocks of 128)
GBLK = G // P                  # 2
S_STEPS = 8                    # CC quadrature steps on the grid
S_REAL = S_STEPS + 1           # 9 quadrature points
S = 12                         # padded to multiple of 4
TILE_F = 512                   # free-dim tile (4 s-chunks of 128)
F_G = S * P                    # 1536 free per grid block
NT = F_G // TILE_F             # 3 tiles per block
NBP = 384                      # padded relu-basis breakpoints (3 chunks)
NCH = NBP // P                 # 3 breakpoint chunks
QT = N_LOC // TILE_F           # 4 query tiles
H_DIM = 2

# blob column layouts (elements)
B2_W0, B2_T1, B2_CW0 = 0, 300, 300 + GBLK * F_G
B2W = B2_CW0 + F_G                                   # [2, 4908] f16
BA_W1, BA_W2, BA_W3 = 0, 300, 603
BAW = BA_W3 + 3 * 2 * S                              # [101, 675] f16
BF_G, BF_B1, BF_B2, BF_CCW, BF_AG, BF_BP, BF_BPN, BF_SC, BF_DT = (
    0, 2, 5, 8, 8 + S, 14 + S, 17 + S, 20 + S, 21 + S)
BFW = BF_DT + 4 * P                                  # [128, 545] f32


def _cc_quadrature(nb):
    lam = np.arange(nb + 1).reshape(-1, 1).astype(np.float64)
    lam = np.cos((lam @ lam.T) * np.pi / nb)
    lam[:, 0] = 0.5
    lam[:, -1] = 0.5 * lam[:, -1]
    lam = lam * 2.0 / nb
    W = np.arange(nb + 1).reshape(-1, 1).astype(np.float64)
    W[np.arange(1, nb + 1, 2)] = 0.0
    W = 2.0 / (1.0 - W ** 2)
    W[0] = 1.0
    W[np.arange(1, nb + 1, 2)] = 0.0
    cc_w = (lam.T @ W).flatten()
    steps = np.cos(np.arange(nb + 1) * np.pi / nb)
    return cc_w, steps


CC_W, CC_STEPS = _cc_quadrature(S_STEPS)
C_PAD = np.zeros(S, np.float64)
C_PAD[:S_REAL] = (CC_STEPS + 1.0) * 0.5
CCW_PAD = np.zeros(S, np.float32)
CCW_PAD[:S_REAL] = CC_W


def build_program():
    nc = bacc.Bacc("TRN2", target_bir_lowering=False, debug=False)

    d_b2 = nc.dram_tensor("blob2", [2, B2W], F16, kind="ExternalInput").ap()
    d_ba = nc.dram_tensor("blob101", [101, BAW], F16, kind="ExternalInput").ap()
    d_bf = nc.dram_tensor("blob128", [P, BFW], F32, kind="ExternalInput").ap()
    d_xq = nc.dram_tensor("xq", [N_LOC], F32, kind="ExternalInput").ap()
    d_y = nc.dram_tensor("y", [2, N_LOC], F32, kind="ExternalOutput").ap()

    with tile.TileContext(nc) as tc, contextlib.ExitStack() as ctx:
        singles = ctx.enter_context(tc.tile_pool(name="singles", bufs=1))
        apool = ctx.enter_context(tc.tile_pool(name="apool", bufs=3))
        tailp = ctx.enter_context(tc.tile_pool(name="tailp", bufs=2))
        smallp = ctx.enter_context(tc.tile_pool(name="smallp", bufs=4))
        ppool = ctx.enter_context(tc.tile_pool(name="ppool", bufs=2, space="PSUM"))
        uvpool = ctx.enter_context(tc.tile_pool(name="uvpool", bufs=2, space="PSUM"))

        # ---- packed input DMAs (issue order = priority) ----
        blob2 = singles.tile([2, B2W], F16, tag="blob2")
        nc.sync.dma_start(out=blob2, in_=d_b2)
        bloba = singles.tile([101, BAW], F16, tag="bloba")
        nc.sync.dma_start(out=bloba, in_=d_ba)
        blobf = singles.tile([P, BFW], F32, tag="blobf")
        nc.sync.dma_start(out=blobf, in_=d_bf)
        xbroad = singles.tile([P, N_LOC], F32, tag="xbroad")
        nc.sync.dma_start(out=xbroad, in_=bass.AP(
            tensor=d_xq.tensor, offset=d_xq.offset,
            ap=[[0, P], d_xq.ap[0]]))

        w0b2 = blob2[:, B2_W0:B2_W0 + 300]
        t1v = [blob2[:, B2_T1 + b * F_G:B2_T1 + (b + 1) * F_G]
               for b in range(GBLK)]
        cw0 = blob2[:, B2_CW0:B2_CW0 + F_G]
        w1t = bloba[0:100, BA_W1:BA_W1 + 300]
        w2t = bloba[0:100, BA_W2:BA_W2 + 303]
        w3cc = bloba[:, BA_W3:BA_W3 + 3 * 2 * S]
        gcol = blobf[:, BF_G:BF_G + GBLK]
        b1 = blobf[0:100, BF_B1:BF_B1 + 3]
        b2p = blobf[0:101, BF_B2:BF_B2 + 3]
        ccwb = blobf[:, BF_CCW:BF_CCW + S]
        alphag = blobf[:, BF_AG:BF_AG + 6]
        bpcol = blobf[:, BF_BP:BF_BP + NCH]
        bpncol = blobf[:, BF_BPN:BF_BPN + NCH]
        scl = blobf[:, BF_SC:BF_SC + 1]
        dtb = blobf[:, BF_DT:BF_DT + 4 * P]

        # ---- persistent SBUF ----
        ident = singles.tile([P, P], F32, tag="ident")
        make_identity(nc, ident)
        xx2 = singles.tile([2, G], F16, tag="xx2")
        nc.vector.memset(xx2, 1.0)
        x2t = singles.tile([GBLK, P], F32, tag="x2t")
        x2th = singles.tile([GBLK, P], F16, tag="x2th")
        x2col = singles.tile([P, GBLK], F32, tag="x2col")
        ygcol = singles.tile([P, 2 * GBLK], F32, tag="ygcol")
        csb = singles.tile([P, 2 * NCH], F32, tag="csb")
        you = singles.tile([2, N_LOC], F32, tag="you")
        r_acc = [singles.tile([P, GBLK], F32, tag=f"racc{k}", name=f"racc{k}")
                 for k in range(3)]
        # relu-basis tiles relu(xq - bp); emitted in batches between grid
        # phases (engines are in-order, so emitting them up front would
        # stall the queue on the large xbroad DMA).
        rch = [singles.tile([P, TILE_F], F32, tag=f"rch{ti}_{j}",
                            name=f"rch{ti}_{j}")
               for ti in range(QT) for j in range(NCH)]

        def emit_rch(lo, hi):
            for idx in range(lo, hi):
                ti, j = divmod(idx, NCH)
                xsl = xbroad[:, ti * TILE_F:(ti + 1) * TILE_F]
                if idx % 2 == 0:
                    nc.vector.tensor_scalar(
                        out=rch[idx], in0=xsl,
                        scalar1=bpcol[:, j:j + 1], scalar2=0.0,
                        op0=mybir.AluOpType.subtract, op1=mybir.AluOpType.max)
                else:
                    nc.scalar.activation(
                        out=rch[idx], in_=xsl,
                        func=mybir.ActivationFunctionType.Relu,
                        bias=bpncol[:, j:j + 1], scale=1.0)

        def uv_even_odd(uv):
            step = uv.ap[1][0]
            even = bass.AP(tensor=uv.tensor, offset=uv.offset,
                           ap=[uv.ap[0], [2 * step, S]])
            odd = bass.AP(tensor=uv.tensor, offset=uv.offset + step,
                          ap=[uv.ap[0], [2 * step, S]])
            return even, odd

        def mono_stream(k, b, t1_ap, a2_on_dve):
            """Generator: one (weight-set k, block b) pass, yielding after
            each instruction so two streams can interleave."""
            sfx = f"_{k}_{b}"
            uv = uvpool.tile([P, 2 * S], F32, tag="uv", name="uv" + sfx)
            for i in range(NT):
                a0ps = ppool.tile([P, TILE_F], F32, tag="a0ps",
                                  name=f"a0ps{sfx}_{i}")
                if t1_ap is not None:
                    nc.tensor.matmul(
                        a0ps[0:100, :], lhsT=w0b2[:, k * 100:(k + 1) * 100],
                        rhs=t1_ap[:, i * TILE_F:(i + 1) * TILE_F],
                        start=True, stop=True)
                    yield
                else:
                    for c in range(4):
                        s = 4 * i + c
                        nc.tensor.matmul(
                            a0ps[:, c * P:(c + 1) * P],
                            lhsT=cw0[:, s * P:(s + 1) * P],
                            rhs=xx2[:, b * P:(b + 1) * P],
                            start=True, stop=True)
                        yield
                a0sb = apool.tile([100, TILE_F], F16, tag="a0sb",
                                  name=f"a0sb{sfx}_{i}")
                nc.scalar.activation(out=a0sb, in_=a0ps[0:100, :],
                                     func=mybir.ActivationFunctionType.Relu,
                                     bias=0.0, scale=1.0)
                yield
                a1ps = ppool.tile([100, TILE_F], F32, tag="a1ps",
                                  name=f"a1ps{sfx}_{i}")
                nc.tensor.matmul(a1ps, lhsT=w1t[:, k * 100:(k + 1) * 100],
                                 rhs=a0sb, start=True, stop=True)
                yield
                a1sb = apool.tile([100, TILE_F], F16, tag="a1sb",
                                  name=f"a1sb{sfx}_{i}")
                nc.vector.tensor_scalar(out=a1sb, in0=a1ps,
                                        scalar1=b1[:, k:k + 1], scalar2=0.0,
                                        op0=mybir.AluOpType.add,
                                        op1=mybir.AluOpType.max)
                yield
                a2ps = ppool.tile([101, TILE_F], F32, tag="a2ps",
                                  name=f"a2ps{sfx}_{i}")
                nc.tensor.matmul(a2ps, lhsT=w2t[:, k * 101:(k + 1) * 101],
                                 rhs=a1sb, start=True, stop=True)
                yield
                a2sb = apool.tile([101, TILE_F], F16, tag="a2sb",
                                  name=f"a2sb{sfx}_{i}")
                if a2_on_dve:
                    nc.vector.tensor_scalar(out=a2sb, in0=a2ps,
                                            scalar1=b2p[:, k:k + 1],
                                            scalar2=0.0,
                                            op0=mybir.AluOpType.add,
                                            op1=mybir.AluOpType.max)
                else:
                    nc.scalar.activation(out=a2sb, in_=a2ps,
                                         func=mybir.ActivationFunctionType.Relu,
                                         bias=b2p[:, k:k + 1], scale=1.0)
                yield
                for c in range(4):
                    s = 4 * i + c
                    nc.tensor.matmul(
                        uv[:, 2 * s:2 * s + 2],
                        lhsT=a2sb[:, c * P:(c + 1) * P],
                        rhs=w3cc[:, k * 2 * S + 2 * s:k * 2 * S + 2 * s + 2],
                        start=True, stop=True)
                    yield
            # tail: r = sum_s relu(u) + sum_s ccw*exp(-relu(v))
            even, odd = uv_even_odd(uv)
            junk = tailp.tile([P, S], F32, tag="junk", name="junk" + sfx)
            r1 = smallp.tile([P, 1], F32, tag="r1", name="r1" + sfx)
            nc.scalar.activation(out=junk, in_=even,
                                 func=mybir.ActivationFunctionType.Relu,
                                 bias=0.0, scale=1.0, accum_out=r1[:, 0:1])
            yield
            wneg = tailp.tile([P, S], F32, tag="wneg", name="wneg" + sfx)
            nc.scalar.activation(out=wneg, in_=odd,
                                 func=mybir.ActivationFunctionType.Relu,
                                 bias=0.0, scale=1.0)
            yield
            e_t = tailp.tile([P, S], F32, tag="e_t", name="e_t" + sfx)
            nc.scalar.activation(out=e_t, in_=wneg,
                                 func=mybir.ActivationFunctionType.Exp,
                                 bias=0.0, scale=-1.0)
            yield
            g_t = tailp.tile([P, S], F32, tag="g_t", name="g_t" + sfx)
            nc.vector.tensor_mul(g_t, e_t, ccwb)
            yield
            r2 = smallp.tile([P, 1], F32, tag="r2", name="r2" + sfx)
            nc.vector.tensor_reduce(out=r2[:, 0:1], in_=g_t,
                                    axis=mybir.AxisListType.X,
                                    op=mybir.AluOpType.add)
            yield
            nc.vector.tensor_add(r_acc[k][:, b:b + 1], r1, r2)

        def drive(*gens):
            live = list(gens)
            while live:
                for g in list(live):
                    try:
                        next(g)
                    except StopIteration:
                        live.remove(g)

        def finalize(k, xcol_ap, out_ap):
            m = smallp.tile([P, GBLK], F32, tag="fin_m", name=f"fin_m{k}")
            nc.vector.tensor_mul(m, xcol_ap, r_acc[k])
            nc.vector.tensor_scalar(out=out_ap, in0=m,
                                    scalar1=alphag[:, k:k + 1],
                                    scalar2=alphag[:, 3 + k:4 + k],
                                    op0=mybir.AluOpType.mult,
                                    op1=mybir.AluOpType.add)

        # ---- grid eval: mono0 first (both blocks interleaved) so the
        # x2 chain and its DMA latency hide under mono1 ----
        drive(mono_stream(0, 0, t1v[0], False),
              mono_stream(0, 1, t1v[1], True))

        # x2 = finalize(mono0); build xx2 row 0 = x2
        finalize(0, gcol, x2col)
        x2t_ps = uvpool.tile([GBLK, P], F32, tag="uv")
        nc.tensor.transpose(x2t_ps, x2col, ident)
        nc.scalar.copy(x2t, x2t_ps)
        nc.vector.tensor_copy(x2th, x2t)
        for b in range(GBLK):
            nc.sync.dma_start(out=xx2[0:1, b * P:(b + 1) * P],
                              in_=x2th[b:b + 1, :])
        emit_rch(0, 6)

        # ---- mono 1 (both blocks interleaved) ----
        drive(mono_stream(1, 0, t1v[0], False),
              mono_stream(1, 1, t1v[1], True))
        # y1 grid values -> ygcol columns 0, 2 (chunk-major, func-minor)
        y1_ap = bass.AP(tensor=ygcol.tensor, offset=ygcol.offset,
                        ap=[ygcol.ap[0], [2 * ygcol.ap[1][0], GBLK]])
        finalize(1, gcol, y1_ap)
        emit_rch(6, 12)

        # ---- mono 2 on x2 grid: both blocks interleaved ----
        drive(mono_stream(2, 0, None, False),
              mono_stream(2, 1, None, True))
        y2_ap = bass.AP(tensor=ygcol.tensor,
                        offset=ygcol.offset + ygcol.ap[1][0],
                        ap=[ygcol.ap[0], [2 * ygcol.ap[1][0], GBLK]])
        finalize(2, x2col, y2_ap)

        # ---- PWL coefficients: c = D @ ygrid (both funcs at once, N=2) ----
        # nonzero D^T blocks: (j=0,i=0), (j=1,i=0), (j=1,i=1), (j=2,i=1)
        blocks = [(0, 0, 0), (1, 1, 0), (2, 1, 1), (3, 2, 1)]
        cps = {}
        for j in range(NCH):
            cps[j] = uvpool.tile([P, 2], F32, tag="uv", name=f"cps{j}")
        for blk, j, i in blocks:
            first = (blk == 0 or blocks[blk - 1][1] != j)
            last = (blk == 3 or blocks[blk + 1][1] != j)
            nc.tensor.matmul(cps[j],
                             lhsT=dtb[:, blk * P:(blk + 1) * P],
                             rhs=ygcol[:, 2 * i:2 * i + 2],
                             start=first, stop=last)
        for j in range(NCH):
            nc.scalar.copy(csb[:, 2 * j:2 * j + 2], cps[j])

        # ---- interpolate queries: y[f, n] = sum_i c[f,i] relu(x_n - bp_i) ----
        for ti in range(QT):
            yps = ppool.tile([P, TILE_F], F32, tag="a0ps", name=f"yps{ti}")
            for j in range(NCH):
                nc.tensor.matmul(yps[0:2, :],
                                 lhsT=csb[:, 2 * j:2 * j + 2],
                                 rhs=rch[ti * NCH + j],
                                 start=(j == 0), stop=(j == NCH - 1))
            # the interp matmul computes 16h*y (integer-exact D blocks);
            # rescale while copying PSUM -> SBUF
            nc.vector.tensor_scalar(
                out=you[:, ti * TILE_F:(ti + 1) * TILE_F], in0=yps[0:2, :],
                scalar1=scl[0:2, 0:1], scalar2=None,
                op0=mybir.AluOpType.mult)
        nc.sync.dma_start(out=d_y, in_=you)

    nc.compile()
    return nc


def host_grid_inputs(x_full, iws, ibs, hws, hbs):
    """Build the shared (grid + weights) input map; xq is added per core."""
    (iW0, iW1, iW2, iW3) = iws
    (ib0, ib1, ib2, ib3) = ibs

    xlo = float(x_full.min()) - 1e-3
    xhi = float(x_full.max()) + 1e-3
    b = np.linspace(xlo, xhi, G)
    h = b[1] - b[0]

    # f16 blob [2, B2W]: w0b2 | t-rows per block | cw0
    blob2 = np.empty((2, B2W), np.float16)
    w0col = iW0[:, :, 0]
    for k in range(3):
        blob2[0, k * 100:(k + 1) * 100] = w0col[k]
        blob2[1, k * 100:(k + 1) * 100] = ib0[k]
    bb = b.reshape(GBLK, P)
    grid = C_PAD[:, None] * bb[:, None, :]           # [blk, s, p]
    blob2[0, B2_T1:B2_CW0] = grid.reshape(GBLK * F_G)
    blob2[1, B2_T1:B2_CW0] = 1.0
    cw = np.zeros((2, F_G), np.float32)
    for s in range(S):
        cw[0, s * P:s * P + 100] = C_PAD[s] * w0col[2]
        cw[1, s * P:s * P + 100] = ib0[2]
    blob2[:, B2_CW0:] = cw

    # f16 blob [101, BAW]: w1t | w2t | w3cc
    bloba = np.zeros((101, BAW), np.float16)
    for k in range(3):
        bloba[0:100, BA_W1 + k * 100:BA_W1 + (k + 1) * 100] = iW1[k].T
        bloba[0:100, BA_W2 + k * 101:BA_W2 + k * 101 + 100] = iW2[k].T
    w3cc2 = np.zeros((101, 3 * 2 * S), np.float32)
    for k in range(3):
        col_p = np.concatenate([iW3[k, 0, :], [ib3[k, 0]]])
        for s in range(S):
            w3cc2[:, k * 2 * S + 2 * s] = CCW_PAD[s] * col_p
            w3cc2[:, k * 2 * S + 2 * s + 1] = -col_p
    bloba[:, BA_W3:] = w3cc2.astype(np.float16)

    # conditioner at h=0: alpha_k = 0.5*exp(c1_k), gamma_k = c0_k
    ag = np.empty(6, np.float64)
    for k in range(3):
        hh = np.zeros(H_DIM, np.float64)
        for li, (W, bv) in enumerate(zip(hws, hbs)):
            hh = W[k].astype(np.float64) @ hh + bv[k].astype(np.float64)
            if li < len(hws) - 1:
                hh = np.maximum(hh, 0.0)
        ag[k] = 0.5 * np.exp(hh[1])
        ag[3 + k] = hh[0]

    # breakpoints + filtered second-difference matrix. The device stores
    # M = 16h * D_filtered, which is exactly integral (entries <= 38), so
    # the f32 matmul keeps the row-sum cancellations exact; the final
    # output copy rescales by 1/(16h). Constant channels use gap 16h.
    bp = np.full(NBP, 1e9, np.float64)
    bp[0] = xlo - 32.0 * h
    bp[1] = xlo - 16.0 * h
    bp[2:2 + G - 1] = b[:G - 1]
    D = np.zeros((NBP, G))
    D[0, 0] = 1.0 / (16.0 * h)
    D[1, 0] = -1.0 / (16.0 * h)
    D[2, 0] = -1.0 / h
    D[2, 1] = 1.0 / h
    for j in range(1, G - 1):
        D[2 + j, j - 1] = 1.0 / h
        D[2 + j, j] = -2.0 / h
        D[2 + j, j + 1] = 1.0 / h
    T = np.zeros((G, G))
    for j in range(1, G - 1):
        T[j, j - 1] = 1.0
        T[j, j] = -2.0
        T[j, j + 1] = 1.0
    D = 16.0 * h * (D @ (np.eye(G) - T / 16.0))
    D = np.round(D)
    assert np.abs(D).max() < 2 ** 22

    # f32 blob [128, BFW]: gcol | b1 | b2p | ccwb | alphag | bpcol | dtb
    blobf = np.zeros((P, BFW), np.float32)
    blobf[:, BF_G:BF_G + GBLK] = b.reshape(GBLK, P).T
    blobf[0:100, BF_B1:BF_B1 + 3] = ib1.T
    for k in range(3):
        blobf[0:100, BF_B2 + k] = ib2[k]
        blobf[100, BF_B2 + k] = 1.0
    blobf[:, BF_CCW:BF_CCW + S] = CCW_PAD[None, :]
    blobf[:, BF_AG:BF_AG + 6] = ag[None, :]
    blobf[:, BF_BP:BF_BP + NCH] = bp.reshape(NCH, P).T
    blobf[:, BF_BPN:BF_BPN + NCH] = -bp.reshape(NCH, P).T
    blobf[:, BF_SC] = 1.0 / (16.0 * h)
    for blk, (j, i) in enumerate(((0, 0), (1, 0), (1, 1), (2, 1))):
        blobf[:, BF_DT + blk * P:BF_DT + (blk + 1) * P] = \
            D[j * P:(j + 1) * P, i * P:(i + 1) * P].T

    return {"blob2": blob2, "blob101": bloba, "blob128": blobf}


def make_in_maps(logits_quality,
                 iW0, ib0, iW1, ib1, iW2, ib2, iW3, ib3,
                 hW0, hb0, hW1, hb1, hW2, hb2, hW3, hb3, **_):
    x = np.asarray(logits_quality, np.float32)
    iws = [np.asarray(a, np.float32) for a in (iW0, iW1, iW2, iW3)]
    ibs = [np.asarray(a, np.float32) for a in (ib0, ib1, ib2, ib3)]
    hws = [np.asarray(a, np.float32) for a in (hW0, hW1, hW2, hW3)]
    hbs = [np.asarray(a, np.float32) for a in (hb0, hb1, hb2, hb3)]
    shared = host_grid_inputs(x, iws, ibs, hws, hbs)
    in_maps = []
    for c in range(N_CORES):
        im = dict(shared)
        im["xq"] = np.ascontiguousarray(x[c * N_LOC:(c + 1) * N_LOC])
        in_maps.append(im)
    return x, in_maps


_PROGRAM_CACHE = {}


def get_program():
    if "nc" not in _PROGRAM_CACHE:
        _PROGRAM_CACHE["nc"] = build_program()
    return _PROGRAM_CACHE["nc"]


def kernel(logits_quality, nn_id,
           iW0, ib0, iW1, ib1, iW2, ib2, iW3, ib3,
           hW0, hb0, hW1, hb1, hW2, hb2, hW3, hb3):
    x, in_maps = make_in_maps(
        logits_quality,
        iW0, ib0, iW1, ib1, iW2, ib2, iW3, ib3,
        hW0, hb0, hW1, hb1, hW2, hb2, hW3, hb3)
    nc = get_program()
    res = run_bass_kernel_spmd(nc, in_maps, core_ids=list(range(N_CORES)))
    y1 = np.concatenate([r["y"][0] for r in res.results])
    y2 = np.concatenate([r["y"][1] for r in res.results])
    return (y1, y2, x)


# revision 22
# speedup vs baseline: 1.6733x; 1.0846x over previous
"""Trainium2 Bass kernel for nn_CLIP_MINN_88210038326221.

Computes, for N=16384 samples x with h=zeros(2):
    x2 = mono(0, x);  y1 = mono(1, x);  y2 = mono(2, x2)
where mono(k, x) integrates elu(MLP_k(cat(t, 0, 0)))+1 over t in [0, x] via
Clenshaw-Curtis quadrature, then applies the (constant, because h=0)
conditioner affine: out = exp(c1_k) * z + c0_k.

Because h is always zero, every output is a scalar 1-D function of the
single input x. The kernel therefore:
  1. evaluates the full mono chain on a G=256-point uniform grid covering
     the x range (CC quadrature with 8 steps -- the functions are integrals
     of positive integrands and extremely smooth; ~1e-4 vs the reference's
     100-step rule),
  2. converts the grid values into piecewise-linear coefficients in the
     relu basis  y(x) = sum_i c_i * relu(x - bp_i)  via a constant
     second-difference matrix (on-device matmul; the matrix also carries a
     -h^2 f''/16 anti-bowing filter that halves the interp error),
  3. evaluates both outputs for all queries with three accumulating
     [128]-chunk f32 matmuls per 512-query tile.

Grid evaluation per weight set k (identical math to the direct kernel):
  t[g,s] = b[g] * c[s],  c[s] = (cos(s*pi/8)+1)/2
  a0 = relu(w0 t + b0)        -> K=2 matmul vs [t; 1] rows
  a1 = relu(W1 a0 + b1)       -> K=100 matmul, bias+relu in DVE
  a2 = relu(W2' a1 + b2')     -> zero row + bias 1 gives a free ones channel
  y3 = w3 . a2 + b3           -> lhsT = a2 chunk [101,128], rhs N=2
  sum_s ccw_s*(elu(y3)+1) = sum_s relu(ccw_s*y3') + sum_s ccw_s*exp(min(y3',0))
  out = exp(c1)*z + c0,  z = 0.5 * x * sum_s ccw_s*dz
mono0 and mono1 (and mono2's two grid blocks) are emitted as two
interleaved instruction streams so the PE never stalls on the
activation chain. Inputs arrive in three packed blob DMAs to keep the
serial DMA-issue cost off the critical path.
All 8 cores run the identical grid evaluation (replicated); the 16384
queries are sharded 2048 per core for the interpolation stage.
"""

import contextlib

import numpy as np

import concourse.bacc as bacc
import concourse.bass as bass
import concourse.mybir as mybir
import concourse.tile as tile
from concourse.bass_utils import run_bass_kernel_spmd
from concourse.masks import make_identity

F32 = mybir.dt.float32
F16 = mybir.dt.float16

N_CORES = 8
N_FULL = 16384
N_LOC = N_FULL // N_CORES      # 2048 queries per core
P = 128
G = 256                        # grid points (2 blocks of 128)
GBLK = G // P                  # 2
S_STEPS = 8                    # CC quadrature steps on the grid
S_REAL = S_STEPS + 1           # 9 quadrature points
S = 12                         # padded to multiple of 4
TILE_F = 512                   # free-dim tile (4 s-chunks of 128)
F_G = S * P                    # 1536 free per grid block
NT = F_G // TILE_F             # 3 tiles per block
NBP = 256                      # relu-basis breakpoints (2 chunks)
NCH = NBP // P                 # 2 breakpoint chunks
QT = N_LOC // TILE_F           # 4 query tiles
H_DIM = 2

# blob column layouts (elements)
B2_W0, B2_T1, B2_CW0 = 0, 300, 300 + GBLK * F_G
B2W = B2_CW0 + F_G                                   # [2, 4908] f16
BA_W1, BA_W2, BA_W3 = 0, 300, 603
BAW = BA_W3 + 3 * 2 * S                              # [101, 675] f16
BF_G, BF_B1, BF_B2, BF_CCW, BF_AG, BF_BP, BF_BPN, BF_SC, BF_DT = (
    0, 2, 5, 8, 8 + S, 14 + S, 16 + S, 18 + S, 19 + S)
BFW = BF_DT + 4 * P                                  # [128, 543] f32


def _cc_quadrature(nb):
    lam = np.arange(nb + 1).reshape(-1, 1).astype(np.float64)
    lam = np.cos((lam @ lam.T) * np.pi / nb)
    lam[:, 0] = 0.5
    lam[:, -1] = 0.5 * lam[:, -1]
    lam = lam * 2.0 / nb
    W = np.arange(nb + 1).reshape(-1, 1).astype(np.float64)
    W[np.arange(1, nb + 1, 2)] = 0.0
    W = 2.0 / (1.0 - W ** 2)
    W[0] = 1.0
    W[np.arange(1, nb + 1, 2)] = 0.0
    cc_w = (lam.T @ W).flatten()
    steps = np.cos(np.arange(nb + 1) * np.pi / nb)
    return cc_w, steps


CC_W, CC_STEPS = _cc_quadrature(S_STEPS)
C_PAD = np.zeros(S, np.float64)
C_PAD[:S_REAL] = (CC_STEPS + 1.0) * 0.5
CCW_PAD = np.zeros(S, np.float32)
CCW_PAD[:S_REAL] = CC_W


def build_program():
    nc = bacc.Bacc("TRN2", target_bir_lowering=False, debug=False)

    d_b2 = nc.dram_tensor("blob2", [2, B2W], F16, kind="ExternalInput").ap()
    d_ba = nc.dram_tensor("blob101", [101, BAW], F16, kind="ExternalInput").ap()
    d_bf = nc.dram_tensor("blob128", [P, BFW], F32, kind="ExternalInput").ap()
    d_xq = nc.dram_tensor("xq", [N_LOC], F32, kind="ExternalInput").ap()
    d_y = nc.dram_tensor("y", [2, N_LOC], F32, kind="ExternalOutput").ap()

    with tile.TileContext(nc) as tc, contextlib.ExitStack() as ctx:
        singles = ctx.enter_context(tc.tile_pool(name="singles", bufs=1))
        apool = ctx.enter_context(tc.tile_pool(name="apool", bufs=3))
        tailp = ctx.enter_context(tc.tile_pool(name="tailp", bufs=2))
        smallp = ctx.enter_context(tc.tile_pool(name="smallp", bufs=4))
        ppool = ctx.enter_context(tc.tile_pool(name="ppool", bufs=2, space="PSUM"))
        uvpool = ctx.enter_context(tc.tile_pool(name="uvpool", bufs=2, space="PSUM"))

        # ---- packed input DMAs (issue order = priority) ----
        blob2 = singles.tile([2, B2W], F16, tag="blob2")
        nc.sync.dma_start(out=blob2, in_=d_b2)
        bloba = singles.tile([101, BAW], F16, tag="bloba")
        nc.sync.dma_start(out=bloba, in_=d_ba)
        blobf = singles.tile([P, BFW], F32, tag="blobf")
        nc.sync.dma_start(out=blobf, in_=d_bf)
        xbroad = singles.tile([P, N_LOC], F32, tag="xbroad")
        nc.sync.dma_start(out=xbroad, in_=bass.AP(
            tensor=d_xq.tensor, offset=d_xq.offset,
            ap=[[0, P], d_xq.ap[0]]))

        w0b2 = blob2[:, B2_W0:B2_W0 + 300]
        t1v = [blob2[:, B2_T1 + b * F_G:B2_T1 + (b + 1) * F_G]
               for b in range(GBLK)]
        cw0 = blob2[:, B2_CW0:B2_CW0 + F_G]
        w1t = bloba[0:100, BA_W1:BA_W1 + 300]
        w2t = bloba[0:100, BA_W2:BA_W2 + 303]
        w3cc = bloba[:, BA_W3:BA_W3 + 3 * 2 * S]
        gcol = blobf[:, BF_G:BF_G + GBLK]
        b1 = blobf[0:100, BF_B1:BF_B1 + 3]
        b2p = blobf[0:101, BF_B2:BF_B2 + 3]
        ccwb = blobf[:, BF_CCW:BF_CCW + S]
        alphag = blobf[:, BF_AG:BF_AG + 6]
        bpcol = blobf[:, BF_BP:BF_BP + NCH]
        bpncol = blobf[:, BF_BPN:BF_BPN + NCH]
        scl = blobf[:, BF_SC:BF_SC + 1]
        dtb = blobf[:, BF_DT:BF_DT + 4 * P]

        # ---- persistent SBUF ----
        ident = singles.tile([P, P], F32, tag="ident")
        make_identity(nc, ident)
        # warm the activation table while input DMAs stream
        warm = singles.tile([1, 4], F32, tag="warm")
        nc.scalar.activation(out=warm, in_=ident[0:1, 0:4],
                             func=mybir.ActivationFunctionType.Relu,
                             bias=0.0, scale=1.0)
        xx2 = singles.tile([2, G], F16, tag="xx2")
        nc.vector.memset(xx2, 1.0)
        x2t = singles.tile([GBLK, P], F32, tag="x2t")
        x2th = singles.tile([GBLK, P], F16, tag="x2th")
        x2col = singles.tile([P, GBLK], F32, tag="x2col")
        ygcol = singles.tile([P, 2 * GBLK], F32, tag="ygcol")
        csb = singles.tile([P, 2 * NCH], F32, tag="csb")
        offt = singles.tile([2, 1], F32, tag="offt")
        you = singles.tile([2, N_LOC], F32, tag="you")
        r_acc = [singles.tile([P, GBLK], F32, tag=f"racc{k}", name=f"racc{k}")
                 for k in range(3)]
        # relu-basis tiles relu(xq - bp); emitted in batches between grid
        # phases (engines are in-order, so emitting them up front would
        # stall the queue on the large xbroad DMA).
        rch = [singles.tile([P, TILE_F], F32, tag=f"rch{ti}_{j}",
                            name=f"rch{ti}_{j}")
               for ti in range(QT) for j in range(NCH)]

        def emit_rch(lo, hi):
            for idx in range(lo, hi):
                ti, j = divmod(idx, NCH)
                xsl = xbroad[:, ti * TILE_F:(ti + 1) * TILE_F]
                if idx % 2 == 0:
                    nc.vector.tensor_scalar(
                        out=rch[idx], in0=xsl,
                        scalar1=bpcol[:, j:j + 1], scalar2=0.0,
                        op0=mybir.AluOpType.subtract, op1=mybir.AluOpType.max)
                else:
                    nc.scalar.activation(
                        out=rch[idx], in_=xsl,
                        func=mybir.ActivationFunctionType.Relu,
                        bias=bpncol[:, j:j + 1], scale=1.0)

        def uv_even_odd(uv):
            step = uv.ap[1][0]
            even = bass.AP(tensor=uv.tensor, offset=uv.offset,
                           ap=[uv.ap[0], [2 * step, S]])
            odd = bass.AP(tensor=uv.tensor, offset=uv.offset + step,
                          ap=[uv.ap[0], [2 * step, S]])
            return even, odd

        def mono_stream(k, b, t1_ap, a2_on_dve):
            """Generator: one (weight-set k, block b) pass, yielding after
            each instruction so two streams can interleave."""
            sfx = f"_{k}_{b}"
            uv = uvpool.tile([P, 2 * S], F32, tag="uv", name="uv" + sfx)
            for i in range(NT):
                a0ps = ppool.tile([P, TILE_F], F32, tag="a0ps",
                                  name=f"a0ps{sfx}_{i}")
                if t1_ap is not None:
                    nc.tensor.matmul(
                        a0ps[0:100, :], lhsT=w0b2[:, k * 100:(k + 1) * 100],
                        rhs=t1_ap[:, i * TILE_F:(i + 1) * TILE_F],
                        start=True, stop=True)
                    yield
                else:
                    for c in range(4):
                        s = 4 * i + c
                        nc.tensor.matmul(
                            a0ps[:, c * P:(c + 1) * P],
                            lhsT=cw0[:, s * P:(s + 1) * P],
                            rhs=xx2[:, b * P:(b + 1) * P],
                            start=True, stop=True)
                        yield
                a0sb = apool.tile([100, TILE_F], F16, tag="a0sb",
                                  name=f"a0sb{sfx}_{i}")
                nc.scalar.activation(out=a0sb, in_=a0ps[0:100, :],
                                     func=mybir.ActivationFunctionType.Relu,
                                     bias=0.0, scale=1.0)
                yield
                a1ps = ppool.tile([100, TILE_F], F32, tag="a1ps",
                                  name=f"a1ps{sfx}_{i}")
                nc.tensor.matmul(a1ps, lhsT=w1t[:, k * 100:(k + 1) * 100],
                                 rhs=a0sb, start=True, stop=True)
                yield
                a1sb = apool.tile([100, TILE_F], F16, tag="a1sb",
                                  name=f"a1sb{sfx}_{i}")
                nc.vector.tensor_scalar(out=a1sb, in0=a1ps,
                                        scalar1=b1[:, k:k + 1], scalar2=0.0,
                                        op0=mybir.AluOpType.add,
                                        op1=mybir.AluOpType.max)
                yield
                a2ps = ppool.tile([101, TILE_F], F32, tag="a2ps",
                                  name=f"a2ps{sfx}_{i}")
                nc.tensor.matmul(a2ps, lhsT=w2t[:, k * 101:(k + 1) * 101],
                                 rhs=a1sb, start=True, stop=True)
                yield
                a2sb = apool.tile([101, TILE_F], F16, tag="a2sb",
                                  name=f"a2sb{sfx}_{i}")
                if a2_on_dve:
                    nc.vector.tensor_scalar(out=a2sb, in0=a2ps,
                                            scalar1=b2p[:, k:k + 1],
                                            scalar2=0.0,
                                            op0=mybir.AluOpType.add,
                                            op1=mybir.AluOpType.max)
                else:
                    nc.scalar.activation(out=a2sb, in_=a2ps,
                                         func=mybir.ActivationFunctionType.Relu,
                                         bias=b2p[:, k:k + 1], scale=1.0)
                yield
                for c in range(4):
                    s = 4 * i + c
                    nc.tensor.matmul(
                        uv[:, 2 * s:2 * s + 2],
                        lhsT=a2sb[:, c * P:(c + 1) * P],
                        rhs=w3cc[:, k * 2 * S + 2 * s:k * 2 * S + 2 * s + 2],
                        start=True, stop=True)
                    yield
            # tail: r = sum_s relu(u) + sum_s ccw*exp(-relu(v))
            even, odd = uv_even_odd(uv)
            junk = tailp.tile([P, S], F32, tag="junk", name="junk" + sfx)
            r1 = smallp.tile([P, 1], F32, tag="r1", name="r1" + sfx)
            nc.scalar.activation(out=junk, in_=even,
                                 func=mybir.ActivationFunctionType.Relu,
                                 bias=0.0, scale=1.0, accum_out=r1[:, 0:1])
            yield
            wneg = tailp.tile([P, S], F32, tag="wneg", name="wneg" + sfx)
            nc.scalar.activation(out=wneg, in_=odd,
                                 func=mybir.ActivationFunctionType.Relu,
                                 bias=0.0, scale=1.0)
            yield
            e_t = tailp.tile([P, S], F32, tag="e_t", name="e_t" + sfx)
            nc.scalar.activation(out=e_t, in_=wneg,
                                 func=mybir.ActivationFunctionType.Exp,
                                 bias=0.0, scale=-1.0)
            yield
            g_t = tailp.tile([P, S], F32, tag="g_t", name="g_t" + sfx)
            nc.vector.tensor_mul(g_t, e_t, ccwb)
            yield
            r2 = smallp.tile([P, 1], F32, tag="r2", name="r2" + sfx)
            nc.vector.tensor_reduce(out=r2[:, 0:1], in_=g_t,
                                    axis=mybir.AxisListType.X,
                                    op=mybir.AluOpType.add)
            yield
            nc.vector.tensor_add(r_acc[k][:, b:b + 1], r1, r2)

        def drive(*gens):
            live = list(gens)
            while live:
                for g in list(live):
                    try:
                        next(g)
                    except StopIteration:
                        live.remove(g)

        def finalize(k, xcol_ap, out_ap):
            m = smallp.tile([P, GBLK], F32, tag="fin_m", name=f"fin_m{k}")
            nc.vector.tensor_mul(m, xcol_ap, r_acc[k])
            nc.vector.tensor_scalar(out=out_ap, in0=m,
                                    scalar1=alphag[:, k:k + 1],
                                    scalar2=alphag[:, 3 + k:4 + k],
                                    op0=mybir.AluOpType.mult,
                                    op1=mybir.AluOpType.add)

        # ---- grid eval: mono0 first (both blocks interleaved) so the
        # x2 chain and its DMA latency hide under mono1 ----
        drive(mono_stream(0, 0, t1v[0], False),
              mono_stream(0, 1, t1v[1], True))

        # x2 = finalize(mono0); build xx2 row 0 = x2
        finalize(0, gcol, x2col)
        x2t_ps = uvpool.tile([GBLK, P], F32, tag="uv")
        nc.tensor.transpose(x2t_ps, x2col, ident)
        nc.scalar.copy(x2t, x2t_ps)
        nc.vector.tensor_copy(x2th, x2t)
        for b in range(GBLK):
            nc.sync.dma_start(out=xx2[0:1, b * P:(b + 1) * P],
                              in_=x2th[b:b + 1, :])
        emit_rch(0, 4)

        # ---- mono 1 (both blocks interleaved) ----
        drive(mono_stream(1, 0, t1v[0], False),
              mono_stream(1, 1, t1v[1], True))
        # y1 grid values -> ygcol columns 0, 2 (chunk-major, func-minor)
        y1_ap = bass.AP(tensor=ygcol.tensor, offset=ygcol.offset,
                        ap=[ygcol.ap[0], [2 * ygcol.ap[1][0], GBLK]])
        finalize(1, gcol, y1_ap)
        emit_rch(4, 8)

        # ---- mono 2 on x2 grid: both blocks interleaved ----
        drive(mono_stream(2, 0, None, False),
              mono_stream(2, 1, None, True))
        y2_ap = bass.AP(tensor=ygcol.tensor,
                        offset=ygcol.offset + ygcol.ap[1][0],
                        ap=[ygcol.ap[0], [2 * ygcol.ap[1][0], GBLK]])
        finalize(2, x2col, y2_ap)

        # offset = grid value at b0 for both funcs: ygcol[0, 0:2] -> [2, 1]
        off_ps = uvpool.tile([2, 1], F32, tag="uv", name="off_ps")
        nc.tensor.transpose(off_ps, ygcol[0:1, 0:2], ident[0:1, 0:1])
        nc.scalar.copy(offt, off_ps)

        # ---- PWL coefficients: c = D @ ygrid (both funcs at once, N=2) ----
        # nonzero D^T blocks: (j=0,i=0), (j=0,i=1), (j=1,i=0), (j=1,i=1)
        blocks = [(0, 0, 0), (1, 0, 1), (2, 1, 0), (3, 1, 1)]
        cps = {}
        for j in range(NCH):
            cps[j] = uvpool.tile([P, 2], F32, tag="uv", name=f"cps{j}")
        for blk, j, i in blocks:
            first = (blk == 0 or blocks[blk - 1][1] != j)
            last = (blk == 3 or blocks[blk + 1][1] != j)
            nc.tensor.matmul(cps[j],
                             lhsT=dtb[:, blk * P:(blk + 1) * P],
                             rhs=ygcol[:, 2 * i:2 * i + 2],
                             start=first, stop=last)
        for j in range(NCH):
            nc.scalar.copy(csb[:, 2 * j:2 * j + 2], cps[j])

        # ---- interpolate queries: y[f, n] = sum_i c[f,i] relu(x_n - bp_i) ----
        for ti in range(QT):
            yps = ppool.tile([P, TILE_F], F32, tag="a0ps", name=f"yps{ti}")
            for j in range(NCH):
                nc.tensor.matmul(yps[0:2, :],
                                 lhsT=csb[:, 2 * j:2 * j + 2],
                                 rhs=rch[ti * NCH + j],
                                 start=(j == 0), stop=(j == NCH - 1))
            # the interp matmul computes 16h*y (integer-exact D blocks);
            # rescale while copying PSUM -> SBUF
            nc.vector.tensor_scalar(
                out=you[:, ti * TILE_F:(ti + 1) * TILE_F], in0=yps[0:2, :],
                scalar1=scl[0:2, 0:1], scalar2=offt[0:2, 0:1],
                op0=mybir.AluOpType.mult, op1=mybir.AluOpType.add)
        nc.sync.dma_start(out=d_y, in_=you)

    nc.compile()
    return nc


def host_grid_inputs(x_full, iws, ibs, hws, hbs):
    """Build the shared (grid + weights) input map; xq is added per core."""
    (iW0, iW1, iW2, iW3) = iws
    (ib0, ib1, ib2, ib3) = ibs

    xlo = float(x_full.min()) - 1e-3
    xhi = float(x_full.max()) + 1e-3
    b = np.linspace(xlo, xhi, G)
    h = b[1] - b[0]

    # f16 blob [2, B2W]: w0b2 | t-rows per block | cw0
    blob2 = np.empty((2, B2W), np.float16)
    w0col = iW0[:, :, 0]
    for k in range(3):
        blob2[0, k * 100:(k + 1) * 100] = w0col[k]
        blob2[1, k * 100:(k + 1) * 100] = ib0[k]
    bb = b.reshape(GBLK, P)
    grid = C_PAD[:, None] * bb[:, None, :]           # [blk, s, p]
    blob2[0, B2_T1:B2_CW0] = grid.reshape(GBLK * F_G)
    blob2[1, B2_T1:B2_CW0] = 1.0
    cw = np.zeros((2, F_G), np.float32)
    for s in range(S):
        cw[0, s * P:s * P + 100] = C_PAD[s] * w0col[2]
        cw[1, s * P:s * P + 100] = ib0[2]
    blob2[:, B2_CW0:] = cw

    # f16 blob [101, BAW]: w1t | w2t | w3cc
    bloba = np.zeros((101, BAW), np.float16)
    for k in range(3):
        bloba[0:100, BA_W1 + k * 100:BA_W1 + (k + 1) * 100] = iW1[k].T
        bloba[0:100, BA_W2 + k * 101:BA_W2 + k * 101 + 100] = iW2[k].T
    w3cc2 = np.zeros((101, 3 * 2 * S), np.float32)
    for k in range(3):
        col_p = np.concatenate([iW3[k, 0, :], [ib3[k, 0]]])
        for s in range(S):
            w3cc2[:, k * 2 * S + 2 * s] = CCW_PAD[s] * col_p
            w3cc2[:, k * 2 * S + 2 * s + 1] = -col_p
    bloba[:, BA_W3:] = w3cc2.astype(np.float16)

    # conditioner at h=0: alpha_k = 0.5*exp(c1_k), gamma_k = c0_k
    ag = np.empty(6, np.float64)
    for k in range(3):
        hh = np.zeros(H_DIM, np.float64)
        for li, (W, bv) in enumerate(zip(hws, hbs)):
            hh = W[k].astype(np.float64) @ hh + bv[k].astype(np.float64)
            if li < len(hws) - 1:
                hh = np.maximum(hh, 0.0)
        ag[k] = 0.5 * np.exp(hh[1])
        ag[3 + k] = hh[0]

    # breakpoints + filtered slope-difference matrix. The device stores
    # M = 16h * D_filtered, which is exactly integral (entries <= 38), so
    # the f32 matmul keeps the row-sum cancellations exact; the final
    # output copy rescales by 1/(16h) and adds the y(b0) offset.
    bp = np.full(NBP, 1e9, np.float64)
    bp[:G - 1] = b[:G - 1]
    T = np.zeros((G, G))
    for j in range(1, G - 1):
        T[j, j - 1] = 1.0
        T[j, j] = -2.0
        T[j, j + 1] = 1.0
    F = np.eye(G) - T / 16.0
    SL = F[1:] - F[:-1]                  # slopes * h
    D = np.zeros((NBP, G))
    D[0] = SL[0]
    for j in range(1, G - 1):
        D[j] = SL[j] - SL[j - 1]
    D = np.round(16.0 * D)
    assert np.abs(D).max() < 2 ** 22

    # f32 blob [128, BFW]: gcol | b1 | b2p | ccwb | alphag | bpcol | dtb
    blobf = np.zeros((P, BFW), np.float32)
    blobf[:, BF_G:BF_G + GBLK] = b.reshape(GBLK, P).T
    blobf[0:100, BF_B1:BF_B1 + 3] = ib1.T
    for k in range(3):
        blobf[0:100, BF_B2 + k] = ib2[k]
        blobf[100, BF_B2 + k] = 1.0
    blobf[:, BF_CCW:BF_CCW + S] = CCW_PAD[None, :]
    blobf[:, BF_AG:BF_AG + 6] = ag[None, :]
    blobf[:, BF_BP:BF_BP + NCH] = bp.reshape(NCH, P).T
    blobf[:, BF_BPN:BF_BPN + NCH] = -bp.reshape(NCH, P).T
    blobf[:, BF_SC] = 1.0 / (16.0 * h)
    for blk, (j, i) in enumerate(((0, 0), (0, 1), (1, 0), (1, 1))):
        blobf[:, BF_DT + blk * P:BF_DT + (blk + 1) * P] = \
            D[j * P:(j + 1) * P, i * P:(i + 1) * P].T

    return {"blob2": blob2, "blob101": bloba, "blob128": blobf}


def make_in_maps(logits_quality,
                 iW0, ib0, iW1, ib1, iW2, ib2, iW3, ib3,
                 hW0, hb0, hW1, hb1, hW2, hb2, hW3, hb3, **_):
    x = np.asarray(logits_quality, np.float32)
    iws = [np.asarray(a, np.float32) for a in (iW0, iW1, iW2, iW3)]
    ibs = [np.asarray(a, np.float32) for a in (ib0, ib1, ib2, ib3)]
    hws = [np.asarray(a, np.float32) for a in (hW0, hW1, hW2, hW3)]
    hbs = [np.asarray(a, np.float32) for a in (hb0, hb1, hb2, hb3)]
    shared = host_grid_inputs(x, iws, ibs, hws, hbs)
    in_maps = []
    for c in range(N_CORES):
        im = dict(shared)
        im["xq"] = np.ascontiguousarray(x[c * N_LOC:(c + 1) * N_LOC])
        in_maps.append(im)
    return x, in_maps


_PROGRAM_CACHE = {}


def get_program():
    if "nc" not in _PROGRAM_CACHE:
        _PROGRAM_CACHE["nc"] = build_program()
    return _PROGRAM_CACHE["nc"]


def kernel(logits_quality, nn_id,
           iW0, ib0, iW1, ib1, iW2, ib2, iW3, ib3,
           hW0, hb0, hW1, hb1, hW2, hb2, hW3, hb3):
    x, in_maps = make_in_maps(
        logits_quality,
        iW0, ib0, iW1, ib1, iW2, ib2, iW3, ib3,
        hW0, hb0, hW1, hb1, hW2, hb2, hW3, hb3)
    nc = get_program()
    res = run_bass_kernel_spmd(nc, in_maps, core_ids=list(range(N_CORES)))
    y1 = np.concatenate([r["y"][0] for r in res.results])
    y2 = np.concatenate([r["y"][1] for r in res.results])
    return (y1, y2, x)
